# revision 5
# baseline (speedup 1.0000x reference)
"""Trainium2 Bass kernel for EnergyConditionedAtomAttention.

Sharding: data-parallel over B across 8 NeuronCores (4 batches/core).

Device dataflow (per core, feature-major activations xT[feat(part), rows(free)]):
  q-MLP L1 is decomposed: q_in = [h_abs | e_feat] row-concat, where the h_abs
  part is constant across the 512 energy rows of a batch. So
    q1_pre = W1e.T @ e_featT (shared over batches)  + per-batch (W1h.T @ h_abs + b1)
  and the per-batch term is a per-partition bias vector fused into the Silu ACT.
  All MLP layers run weights-stationary: out[dout_tile, rows] = W.T @ xT, which
  chains without transposes. v's last layer runs activation-stationary to get
  row-major v[n, hd] for the attention value matmul. Scores are computed
  directly transposed, sT[n, e] = kT_h.T @ qT_h, so softmax-exp feeds the
  attention matmul with no transpose; the denominator comes from a ones-matmul
  and is applied post-hoc (attn@v)/denom, with v's bias folded in after the
  divide (sum_n attn = 1).

All matmul operands fp16 (full PE rate), PSUM accumulation fp32, biases and
softmax chain fp32. Expected end-to-end rel error vs fp32 reference ~2e-4.
"""
import numpy as np
from contextlib import ExitStack

import concourse.bass as bass
import concourse.tile as tile
from concourse import bacc, mybir
from concourse import bass_utils

# ---- problem constants (hardcoded per contract) ----
B, N, NE = 32, 128, 512
ATOM_DIM, E_DIM, RBF_DIM, HIDDEN, LATENT = 256, 128, 64, 1024, 512
CUTOFF, MAX_Z, ZEMB, NHEADS = 5.0, 100, 64, 8
HEAD_DIM = LATENT // NHEADS  # 64
ATOM_STATIC = ATOM_DIM + ZEMB + RBF_DIM + 3 + 1  # 388
N_CORES = 8
BPC = B // N_CORES  # 4 batches per core

KSTAT = 512  # atom_static padded to 4 K-tiles
F16, F32 = mybir.dt.float16, mybir.dt.float32
Silu = mybir.ActivationFunctionType.Silu
Exp = mybir.ActivationFunctionType.Exp


def _mm_loop(nc, pp, wt, xt, kt_n, mt_n, dout_tile, rows, out_cb, tag="mlp"):
    """out[m] = sum_k  wt[:,k,m-slice].T @ xt[:,k,:rows];  out_cb(m, psum_ap)."""
    for m in range(mt_n):
        p = pp.tile([128, rows], F32, tag=tag)
        for k in range(kt_n):
            nc.tensor.matmul(
                p[:dout_tile, :],
                wt[:, k, m * dout_tile:(m + 1) * dout_tile],
                xt[:, k, 0:rows],
                start=(k == 0), stop=(k == kt_n - 1),
            )
        out_cb(m, p)


def _emit(nc, tc, ctx, D):
    sb = ctx.enter_context(tc.tile_pool(name="sb", bufs=1))
    wp = ctx.enter_context(tc.tile_pool(name="w", bufs=2))
    ap = ctx.enter_context(tc.tile_pool(name="acts", bufs=3))
    pq = ctx.enter_context(tc.tile_pool(name="persist", bufs=1))
    pp = ctx.enter_context(tc.tile_pool(name="ps", bufs=2, space="PSUM"))
    ph = ctx.enter_context(tc.tile_pool(name="psh", bufs=2, space="PSUM"))

    # ---- constants / small inputs ----
    e_feat = sb.tile([128, NE], F16, tag="e_feat")
    nc.sync.dma_start(e_feat[:], D["e_featT"][:])
    h_abs = sb.tile([128, 2, BPC], F16, tag="h_abs")
    nc.sync.dma_start(h_abs[:], D["h_absT"].rearrange("k p b -> p k b"))
    biasn = sb.tile([128, BPC], F32, tag="biasn")
    nc.sync.dma_start(biasn[:], D["bias_n"].rearrange("b p -> p b"))
    bv3 = sb.tile([64, NHEADS], F32, tag="bv3")
    nc.sync.dma_start(bv3[:], D["b_v3"].rearrange("h d -> d h"))
    ones = sb.tile([128, 64], F16, tag="ones")
    nc.vector.memset(ones[:], 1.0)

    def load_w(name, kt_n, dout, tag):
        t = wp.tile([128, kt_n, dout], F16, tag=tag)
        nc.sync.dma_start(t[:], D[name].rearrange("k p d -> p k d"))
        return t

    def load_b(name, mt_n):
        t = sb.tile([128, mt_n], F32, tag=name)
        nc.sync.dma_start(t[:], D[name].rearrange("m p -> p m"))
        return t

    w_q1e = load_w("w_q1e", 1, HIDDEN, "wq1e")
    w_q1h = load_w("w_q1h", 2, HIDDEN, "wq1h")
    b_q1 = load_b("b_q1", 8)
    w_q2 = load_w("w_q2", 8, HIDDEN, "w81")
    b_q2 = load_b("b_q2", 8)
    w_q3 = load_w("w_q3", 8, LATENT, "w85")
    b_q3 = load_b("b_q3", 4)

    # ---- q L1: shared e-part + per-batch h-part as bias ----
    # hterm[1024, b] = W1h.T @ h_absT + b_q1
    hterm = sb.tile([128, 8, BPC], F32, tag="hterm")
    for m in range(8):
        p = ph.tile([128, BPC], F32, tag="hterm")
        for k in range(2):
            nc.tensor.matmul(p[:], w_q1h[:, k, bass.ts(m, 128)], h_abs[:, k, :],
                             start=(k == 0), stop=(k == 1))
        nc.vector.tensor_scalar_add(hterm[:, m, :], p[:], b_q1[:, m:m + 1])
    # q1_pre[1024, 512] = W1e.T @ e_featT   (shared across batches)
    q1pre = sb.tile([128, 8, NE], F16, tag="q1pre")
    def q1pre_cb(m, p):
        nc.vector.tensor_copy(q1pre[:, m, :], p[:])
    _mm_loop(nc, pp, w_q1e, e_feat.rearrange("p (o n) -> p o n", o=1),
             1, 8, 128, NE, q1pre_cb)

    # ---- q L2/L3 per batch ----
    qT = []
    for b in range(BPC):
        q1 = ap.tile([128, 8, NE], F16, tag="a8")
        for m in range(8):
            nc.scalar.activation(q1[:, m, :], q1pre[:, m, :], Silu,
                                 bias=hterm[:, m, b:b + 1])
        q2 = ap.tile([128, 8, NE], F16, tag="a8")
        def q2_cb(m, p):
            nc.scalar.activation(q2[:, m, :], p[:], Silu, bias=b_q2[:, m:m + 1])
        _mm_loop(nc, pp, w_q2, q1, 8, 8, 128, NE, q2_cb)
        qb = pq.tile([128, 4, NE], F16, tag=f"qT{b}")
        def q3_cb(m, p):
            nc.vector.tensor_scalar_add(qb[:, m, :], p[:], b_q3[:, m:m + 1])
        _mm_loop(nc, pp, w_q3, q2, 8, 4, 128, NE, q3_cb)
        qT.append(qb)

    # ---- k/v MLPs on all 4 batches jointly (cols = b*128+n) ----
    atom = sb.tile([128, 4, NE], F16, tag="atom")
    nc.sync.dma_start(atom[:], D["atomT"].rearrange("k p c -> p k c"))

    w_k1 = load_w("w_k1", 4, HIDDEN, "w41")
    b_k1 = load_b("b_k1", 8)
    w_k2 = load_w("w_k2", 8, HIDDEN, "w81")
    b_k2 = load_b("b_k2", 8)
    w_k3 = load_w("w_k3", 8, LATENT, "w85")
    b_k3 = load_b("b_k3", 4)

    def mlp2(xt, w1, b1, w2, b2, kt1):
        y1 = ap.tile([128, 8, NE], F16, tag="a8")
        def l1_cb(m, p):
            nc.scalar.activation(y1[:, m, :], p[:], Silu, bias=b1[:, m:m + 1])
        _mm_loop(nc, pp, w1, xt, kt1, 8, 128, NE, l1_cb)
        y2 = ap.tile([128, 8, NE], F16, tag="a8")
        def l2_cb(m, p):
            nc.scalar.activation(y2[:, m, :], p[:], Silu, bias=b2[:, m:m + 1])
        _mm_loop(nc, pp, w2, y1, 8, 8, 128, NE, l2_cb)
        return y2

    k2t = mlp2(atom, w_k1, b_k1, w_k2, b_k2, 4)
    kT = pq.tile([128, 4, NE], F16, tag="kT")
    def k3_cb(m, p):
        nc.vector.tensor_scalar_add(kT[:, m, :], p[:], b_k3[:, m:m + 1])
    _mm_loop(nc, pp, w_k3, k2t, 8, 4, 128, NE, k3_cb)

    w_v1 = load_w("w_v1", 4, HIDDEN, "w41")
    b_v1 = load_b("b_v1", 8)
    w_v2 = load_w("w_v2", 8, HIDDEN, "w81")
    b_v2 = load_b("b_v2", 8)
    w_v3 = load_w("w_v3", 8, LATENT, "w85")

    v2t = mlp2(atom, w_v1, b_v1, w_v2, b_v2, 4)
    # v L3 activation-stationary -> row-major v[n, hd] per batch (bias folded in later)
    v_sb = pq.tile([128, BPC, LATENT], F16, tag="v_sb")
    for b in range(BPC):
        p = pp.tile([128, LATENT], F32, tag="mlp")
        for k in range(8):
            nc.tensor.matmul(p[:], v2t[:, k, bass.ts(b, 128)], w_v3[:, k, :],
                             start=(k == 0), stop=(k == 7))
        nc.vector.tensor_copy(v_sb[:, b, :], p[:])

    w_o1 = load_w("w_o1", 4, HIDDEN, "w41")
    b_o1 = load_b("b_o1", 8)
    w_o2 = load_w("w_o2", 8, LATENT, "w85")
    b_o2 = load_b("b_o2", 4)

    # ---- attention + o-MLP per batch ----
    for b in range(BPC):
        attn_in = ap.tile([128, 4, NE], F16, tag="attn_in")
        for h in range(NHEADS):
            pb, kt_i = 64 * (h % 2), h // 2
            # sT[n, e] = kT_h.T @ qT_h
            ps_s = pp.tile([128, NE], F32, tag="mlp")
            nc.tensor.matmul(ps_s[:], kT[pb:pb + 64, kt_i, bass.ts(b, 128)],
                             qT[b][pb:pb + 64, kt_i, :],
                             start=True, stop=True, tile_position=(pb, 0))
            # P = exp(s + bias_n)
            p_sb = ap.tile([128, NE], F16, tag="p")
            nc.scalar.activation(p_sb[:], ps_s[:], Exp, bias=biasn[:, b:b + 1])
            # att[hd, e] = v_h.T @ P ; den[e] = 1.T @ P (replicated on 64 parts)
            ps_a = ph.tile([64, NE], F32, tag="att")
            nc.tensor.matmul(ps_a[:], v_sb[:, b, bass.ts(h, 64)], p_sb[:],
                             start=True, stop=True)
            ps_d = ph.tile([64, NE], F32, tag="den")
            nc.tensor.matmul(ps_d[:], ones[:], p_sb[:], start=True, stop=True)
            rec = ap.tile([64, NE], F32, tag="rec")
            nc.vector.reciprocal(rec[:], ps_d[:])
            anorm = ap.tile([64, NE], F32, tag="anorm")
            nc.vector.tensor_tensor(anorm[:], ps_a[:], rec[:], mybir.AluOpType.mult)
            nc.vector.tensor_scalar_add(attn_in[pb:pb + 64, kt_i, :], anorm[:],
                                        bv3[:, h:h + 1])
        # o-MLP
        o1 = ap.tile([128, 8, NE], F16, tag="a8")
        def o1_cb(m, p):
            nc.scalar.activation(o1[:, m, :], p[:], Silu, bias=b_o1[:, m:m + 1])
        _mm_loop(nc, pp, w_o1, attn_in, 4, 8, 128, NE, o1_cb)
        out_sb = ap.tile([128, 4, NE], F32, tag="out32")
        def o2_cb(m, p):
            nc.vector.tensor_scalar_add(out_sb[:, m, :], p[:], b_o2[:, m:m + 1])
        _mm_loop(nc, pp, w_o2, o1, 8, 4, 128, NE, o2_cb)
        nc.sync.dma_start(D["out"][b].rearrange("m p e -> p m e"), out_sb[:])


_CACHE = {}


def _build():
    if "nc" in _CACHE:
        return _CACHE["nc"], _CACHE["names"]
    nc = bacc.Bacc("TRN2", target_bir_lowering=False, debug=False,
                   num_devices=N_CORES)
    D = {}

    def din(name, shape, dt):
        D[name] = nc.dram_tensor(name, shape, dt, kind="ExternalInput").ap()

    din("e_featT", [128, NE], F16)
    din("h_absT", [2, 128, BPC], F16)
    din("bias_n", [BPC, 128], F32)
    din("b_v3", [NHEADS, 64], F32)
    din("atomT", [4, 128, NE], F16)
    for nm, kt_n, dout in [
        ("w_q1e", 1, HIDDEN), ("w_q1h", 2, HIDDEN), ("w_q2", 8, HIDDEN),
        ("w_q3", 8, LATENT), ("w_k1", 4, HIDDEN), ("w_k2", 8, HIDDEN),
        ("w_k3", 8, LATENT), ("w_v1", 4, HIDDEN), ("w_v2", 8, HIDDEN),
        ("w_v3", 8, LATENT), ("w_o1", 4, HIDDEN), ("w_o2", 8, LATENT),
    ]:
        din(nm, [kt_n, 128, dout], F16)
    for nm, mt_n in [("b_q1", 8), ("b_q2", 8), ("b_q3", 4), ("b_k1", 8),
                     ("b_k2", 8), ("b_k3", 4), ("b_v1", 8), ("b_v2", 8),
                     ("b_o1", 8), ("b_o2", 4)]:
        din(nm, [mt_n, 128], F32)
    D["out"] = nc.dram_tensor("out", [BPC, 4, 128, NE], F32,
                              kind="ExternalOutput").ap()

    with tile.TileContext(nc) as tc, ExitStack() as ctx:
        _emit(nc, tc, ctx, D)
    nc.compile()
    names = [k for k in D if k != "out"]
    _CACHE["nc"] = nc
    _CACHE["names"] = names
    return nc, names


def _pad_w(W, kt_n):
    """[din, dout] fp32 -> [kt_n, 128, dout] fp16, K zero-padded."""
    din, dout = W.shape
    Wp = np.zeros((kt_n * 128, dout), np.float16)
    Wp[:din] = W.astype(np.float16)
    return Wp.reshape(kt_n, 128, dout)


def _prep_maps(h, z, pos, mask, e_feat, params, absorber_index):
    ai = int(absorber_index)
    h = np.asarray(h, np.float32)
    z = np.asarray(z)
    pos = np.asarray(pos, np.float32)
    mask = np.asarray(mask)
    e_feat = np.asarray(e_feat, np.float32)
    P = {k: [(np.asarray(W), np.asarray(bb)) for W, bb in v] if k != 'z_emb'
         else np.asarray(v) for k, v in params.items()}

    # geometry + static atom features (host: O(B*N) data prep)
    rel = pos - pos[:, ai, :][:, None, :]
    r = np.sqrt((rel * rel).sum(-1))
    u = rel / np.maximum(r, 1e-8)[..., None]
    valid = mask & (r <= CUTOFF)
    zr = P['z_emb'][z]
    rc = np.minimum(r, CUTOFF)
    centers = np.linspace(0.0, CUTOFF, RBF_DIM).astype(np.float32)
    delta = CUTOFF / (RBF_DIM - 1)
    gamma = 1.0 / (delta * delta + 1e-12)
    rr = np.exp(-gamma * (rc[..., None] - centers) ** 2)
    is_abs = np.zeros_like(r)
    is_abs[:, ai] = 1.0
    atom_static = np.concatenate(
        [h, zr, rr, u, is_abs[..., None]], axis=-1).astype(np.float32)  # [B,N,388]
    cut = 0.5 * (np.cos(np.pi * r / CUTOFF) + 1.0) * (r <= CUTOFF)
    radial = np.log(np.maximum(cut, 1e-8)).astype(np.float32)
    bias_n = np.where(valid, radial, np.float32(-1e9)).astype(np.float32)  # [B,N]

    # shared (per-core-identical) tensors
    shared = {}
    shared["e_featT"] = e_feat.T.astype(np.float16).copy()
    (Wq1, bq1), (Wq2, bq2), (Wq3, bq3) = P['q']
    shared["w_q1h"] = _pad_w(Wq1[:ATOM_DIM], 2)
    shared["w_q1e"] = _pad_w(Wq1[ATOM_DIM:], 1)
    shared["b_q1"] = bq1.astype(np.float32).reshape(8, 128)
    shared["w_q2"] = _pad_w(Wq2, 8)
    shared["b_q2"] = bq2.astype(np.float32).reshape(8, 128)
    shared["w_q3"] = _pad_w(Wq3 * np.float32(HEAD_DIM ** -0.5), 8)
    shared["b_q3"] = (bq3 * np.float32(HEAD_DIM ** -0.5)).astype(np.float32).reshape(4, 128)
    for nm, plist in [("k", P['k']), ("v", P['v'])]:
        (W1, b1), (W2, b2), (W3, b3) = plist
        shared[f"w_{nm}1"] = _pad_w(W1, 4)
        shared[f"b_{nm}1"] = b1.astype(np.float32).reshape(8, 128)
        shared[f"w_{nm}2"] = _pad_w(W2, 8)
        shared[f"b_{nm}2"] = b2.astype(np.float32).reshape(8, 128)
        shared[f"w_{nm}3"] = _pad_w(W3, 8)
        if nm == "k":
            shared["b_k3"] = b3.astype(np.float32).reshape(4, 128)
        else:
            shared["b_v3"] = b3.astype(np.float32).reshape(NHEADS, 64)
    (Wo1, bo1), (Wo2, bo2) = P['o']
    shared["w_o1"] = _pad_w(Wo1, 4)
    shared["b_o1"] = bo1.astype(np.float32).reshape(8, 128)
    shared["w_o2"] = _pad_w(Wo2, 8)
    shared["b_o2"] = bo2.astype(np.float32).reshape(4, 128)

    in_maps = []
    for c in range(N_CORES):
        bs = slice(c * BPC, (c + 1) * BPC)
        m = dict(shared)
        # atom_staticT: [feat(512 pad), cols=(b_local*128+n)] -> [4,128,512]
        a = atom_static[bs]                       # [4,128,388]
        aT = np.zeros((KSTAT, BPC * N), np.float16)
        aT[:ATOM_STATIC] = a.reshape(BPC * N, ATOM_STATIC).T
        m["atomT"] = aT.reshape(4, 128, BPC * N)
        m["h_absT"] = np.ascontiguousarray(
            h[bs, ai, :].T.astype(np.float16)).reshape(2, 128, BPC)
        m["bias_n"] = np.ascontiguousarray(bias_n[bs])  # [4,128]
        in_maps.append(m)
    return in_maps


def kernel(h, z, pos, mask, e_feat, params, absorber_index):
    nc, _ = _build()
    in_maps = _prep_maps(h, z, pos, mask, e_feat, params, absorber_index)
    res = bass_utils.run_bass_kernel_spmd(nc, in_maps, core_ids=list(range(N_CORES)))
    outs = []
    for c in range(N_CORES):
        o = res.results[c]["out"]           # [BPC, 4, 128, NE] = [b, m, d_part, e]
        o = o.reshape(BPC, LATENT, NE).transpose(0, 2, 1)  # [b, e, latent]
        outs.append(o)
    return np.ascontiguousarray(np.concatenate(outs, axis=0).astype(np.float32))


# revision 8
# speedup vs baseline: 14.9371x; 14.9371x over previous
"""Trainium2 Bass kernel for EnergyConditionedAtomAttention.

Sharding: data-parallel over B across 8 NeuronCores (4 batches/core).

Device dataflow (per core, feature-major activations xT[feat(part), rows(free)]):
  q-MLP L1 is decomposed: q_in = [h_abs | e_feat] row-concat, where the h_abs
  part is constant across the 512 energy rows of a batch. So
    q1_pre = W1e.T @ e_featT (shared over batches)  + per-batch (W1h.T @ h_abs + b1)
  and the per-batch term is a per-partition bias vector fused into the Silu ACT.
  All MLP layers run weights-stationary: out[dout_tile, rows] = W.T @ xT, which
  chains without transposes. v's last layer runs activation-stationary to get
  row-major v[n, hd] for the attention value matmul. Scores are computed
  directly transposed, sT[n, e] = kT_h.T @ qT_h, so softmax-exp feeds the
  attention matmul with no transpose; the denominator comes from a ones-matmul
  and is applied post-hoc (attn@v)/denom, with v's bias folded in after the
  divide (sum_n attn = 1).

All matmul operands fp16 (full PE rate), PSUM accumulation fp32, biases and
softmax chain fp32. Expected end-to-end rel error vs fp32 reference ~2e-4.
"""
import numpy as np
from contextlib import ExitStack

import concourse.bass as bass
import concourse.tile as tile
from concourse import bacc, mybir
from concourse import bass_utils

# ---- problem constants (hardcoded per contract) ----
B, N, NE = 32, 128, 512
ATOM_DIM, E_DIM, RBF_DIM, HIDDEN, LATENT = 256, 128, 64, 1024, 512
CUTOFF, MAX_Z, ZEMB, NHEADS = 5.0, 100, 64, 8
HEAD_DIM = LATENT // NHEADS  # 64
ATOM_STATIC = ATOM_DIM + ZEMB + RBF_DIM + 3 + 1  # 388
N_CORES = 8
BPC = B // N_CORES  # 4 batches per core

KSTAT = 512  # atom_static padded to 4 K-tiles
F16, F32 = mybir.dt.float16, mybir.dt.float32
Silu = mybir.ActivationFunctionType.Silu
Exp = mybir.ActivationFunctionType.Exp


def _mm_loop(nc, pp, wt, xt, kt_n, mt_n, dout_tile, rows, out_cb, tag="mlp"):
    """out[m] = sum_k  wt[:,k,m-slice].T @ xt[:,k,:rows];  out_cb(m, psum_ap)."""
    for m in range(mt_n):
        p = pp.tile([128, rows], F32, tag=tag)
        for k in range(kt_n):
            nc.tensor.matmul(
                p[:dout_tile, :],
                wt[:, k, m * dout_tile:(m + 1) * dout_tile],
                xt[:, k, 0:rows],
                start=(k == 0), stop=(k == kt_n - 1),
            )
        out_cb(m, p)


def _emit(nc, tc, ctx, D):
    sb = ctx.enter_context(tc.tile_pool(name="sb", bufs=1))
    wp = ctx.enter_context(tc.tile_pool(name="w", bufs=2))
    ap = ctx.enter_context(tc.tile_pool(name="acts", bufs=3))
    pq = ctx.enter_context(tc.tile_pool(name="persist", bufs=1))
    pp = ctx.enter_context(tc.tile_pool(name="ps", bufs=2, space="PSUM"))
    ph = ctx.enter_context(tc.tile_pool(name="psh", bufs=2, space="PSUM"))

    # ---- constants / small inputs ----
    e_feat = sb.tile([128, NE], F16, tag="e_feat")
    nc.sync.dma_start(e_feat[:], D["e_featT"][:])
    h_abs = sb.tile([128, 2, BPC], F16, tag="h_abs")
    nc.sync.dma_start(h_abs[:], D["h_absT"].rearrange("k p b -> p k b"))
    biasn = sb.tile([128, BPC], F32, tag="biasn")
    nc.sync.dma_start(biasn[:], D["bias_n"].rearrange("b p -> p b"))
    bv3 = sb.tile([64, NHEADS], F32, tag="bv3")
    nc.sync.dma_start(bv3[:], D["b_v3"].rearrange("h d -> d h"))
    ones = sb.tile([128, 64], F16, tag="ones")
    nc.vector.memset(ones[:], 1.0)

    def load_w(name, kt_n, dout, tag):
        t = wp.tile([128, kt_n, dout], F16, tag=tag)
        nc.sync.dma_start(t[:], D[name].rearrange("k p d -> p k d"))
        return t

    def load_b(name, mt_n):
        t = sb.tile([128, mt_n], F32, tag=name)
        nc.sync.dma_start(t[:], D[name].rearrange("m p -> p m"))
        return t

    w_q1e = load_w("w_q1e", 1, HIDDEN, "wq1e")
    w_q1h = load_w("w_q1h", 2, HIDDEN, "wq1h")
    b_q1 = load_b("b_q1", 8)
    w_q2 = load_w("w_q2", 8, HIDDEN, "w81")
    b_q2 = load_b("b_q2", 8)
    w_q3 = load_w("w_q3", 8, LATENT, "w85")
    b_q3 = load_b("b_q3", 4)

    # ---- q L1: shared e-part + per-batch h-part as bias ----
    # hterm[1024, b] = W1h.T @ h_absT + b_q1
    hterm = sb.tile([128, 8, BPC], F32, tag="hterm")
    for m in range(8):
        p = ph.tile([128, BPC], F32, tag="hterm")
        for k in range(2):
            nc.tensor.matmul(p[:], w_q1h[:, k, bass.ts(m, 128)], h_abs[:, k, :],
                             start=(k == 0), stop=(k == 1))
        nc.vector.tensor_scalar_add(hterm[:, m, :], p[:], b_q1[:, m:m + 1])
    # q1_pre[1024, 512] = W1e.T @ e_featT   (shared across batches)
    q1pre = sb.tile([128, 8, NE], F16, tag="q1pre")
    def q1pre_cb(m, p):
        nc.vector.tensor_copy(q1pre[:, m, :], p[:])
    _mm_loop(nc, pp, w_q1e, e_feat.rearrange("p (o n) -> p o n", o=1),
             1, 8, 128, NE, q1pre_cb)

    # ---- q L2/L3 per batch ----
    qT = []
    for b in range(BPC):
        q1 = ap.tile([128, 8, NE], F16, tag="a8")
        for m in range(8):
            nc.scalar.activation(q1[:, m, :], q1pre[:, m, :], Silu,
                                 bias=hterm[:, m, b:b + 1])
        q2 = ap.tile([128, 8, NE], F16, tag="a8")
        def q2_cb(m, p):
            nc.scalar.activation(q2[:, m, :], p[:], Silu, bias=b_q2[:, m:m + 1])
        _mm_loop(nc, pp, w_q2, q1, 8, 8, 128, NE, q2_cb)
        qb = pq.tile([128, 4, NE], F16, tag=f"qT{b}")
        def q3_cb(m, p):
            nc.vector.tensor_scalar_add(qb[:, m, :], p[:], b_q3[:, m:m + 1])
        _mm_loop(nc, pp, w_q3, q2, 8, 4, 128, NE, q3_cb)
        qT.append(qb)

    # ---- k/v MLPs on all 4 batches jointly (cols = b*128+n) ----
    atom = sb.tile([128, 4, NE], F16, tag="atom")
    nc.sync.dma_start(atom[:], D["atomT"].rearrange("k p c -> p k c"))

    w_k1 = load_w("w_k1", 4, HIDDEN, "w41")
    b_k1 = load_b("b_k1", 8)
    w_k2 = load_w("w_k2", 8, HIDDEN, "w81")
    b_k2 = load_b("b_k2", 8)
    w_k3 = load_w("w_k3", 8, LATENT, "w85")
    b_k3 = load_b("b_k3", 4)

    def mlp2(xt, w1, b1, w2, b2, kt1):
        y1 = ap.tile([128, 8, NE], F16, tag="a8")
        def l1_cb(m, p):
            nc.scalar.activation(y1[:, m, :], p[:], Silu, bias=b1[:, m:m + 1])
        _mm_loop(nc, pp, w1, xt, kt1, 8, 128, NE, l1_cb)
        y2 = ap.tile([128, 8, NE], F16, tag="a8")
        def l2_cb(m, p):
            nc.scalar.activation(y2[:, m, :], p[:], Silu, bias=b2[:, m:m + 1])
        _mm_loop(nc, pp, w2, y1, 8, 8, 128, NE, l2_cb)
        return y2

    k2t = mlp2(atom, w_k1, b_k1, w_k2, b_k2, 4)
    kT = pq.tile([128, 4, NE], F16, tag="kT")
    def k3_cb(m, p):
        nc.vector.tensor_scalar_add(kT[:, m, :], p[:], b_k3[:, m:m + 1])
    _mm_loop(nc, pp, w_k3, k2t, 8, 4, 128, NE, k3_cb)

    w_v1 = load_w("w_v1", 4, HIDDEN, "w41")
    b_v1 = load_b("b_v1", 8)
    w_v2 = load_w("w_v2", 8, HIDDEN, "w81")
    b_v2 = load_b("b_v2", 8)
    w_v3 = load_w("w_v3", 8, LATENT, "w85")

    v2t = mlp2(atom, w_v1, b_v1, w_v2, b_v2, 4)
    # v L3 activation-stationary -> row-major v[n, hd] per batch (bias folded in later)
    v_sb = pq.tile([128, BPC, LATENT], F16, tag="v_sb")
    for b in range(BPC):
        p = pp.tile([128, LATENT], F32, tag="mlp")
        for k in range(8):
            nc.tensor.matmul(p[:], v2t[:, k, bass.ts(b, 128)], w_v3[:, k, :],
                             start=(k == 0), stop=(k == 7))
        nc.vector.tensor_copy(v_sb[:, b, :], p[:])

    w_o1 = load_w("w_o1", 4, HIDDEN, "w41")
    b_o1 = load_b("b_o1", 8)
    w_o2 = load_w("w_o2", 8, LATENT, "w85")
    b_o2 = load_b("b_o2", 4)

    # ---- attention + o-MLP per batch ----
    for b in range(BPC):
        attn_in = ap.tile([128, 4, NE], F16, tag="attn_in")
        for h in range(NHEADS):
            pb, kt_i = 64 * (h % 2), h // 2
            # sT[n, e] = kT_h.T @ qT_h
            ps_s = pp.tile([128, NE], F32, tag="mlp")
            nc.tensor.matmul(ps_s[:], kT[pb:pb + 64, kt_i, bass.ts(b, 128)],
                             qT[b][pb:pb + 64, kt_i, :],
                             start=True, stop=True, tile_position=(pb, 0))
            # P = exp(s + bias_n)
            p_sb = ap.tile([128, NE], F16, tag="p")
            nc.scalar.activation(p_sb[:], ps_s[:], Exp, bias=biasn[:, b:b + 1])
            # att[hd, e] = v_h.T @ P ; den[e] = 1.T @ P (replicated on 64 parts)
            ps_a = ph.tile([64, NE], F32, tag="att")
            nc.tensor.matmul(ps_a[:], v_sb[:, b, bass.ts(h, 64)], p_sb[:],
                             start=True, stop=True)
            ps_d = ph.tile([64, NE], F32, tag="den")
            nc.tensor.matmul(ps_d[:], ones[:], p_sb[:], start=True, stop=True)
            rec = ap.tile([64, NE], F32, tag="rec")
            nc.vector.reciprocal(rec[:], ps_d[:])
            anorm = ap.tile([64, NE], F32, tag="anorm")
            nc.vector.tensor_tensor(anorm[:], ps_a[:], rec[:], mybir.AluOpType.mult)
            nc.vector.tensor_scalar_add(attn_in[pb:pb + 64, kt_i, :], anorm[:],
                                        bv3[:, h:h + 1])
        # o-MLP
        o1 = ap.tile([128, 8, NE], F16, tag="a8")
        def o1_cb(m, p):
            nc.scalar.activation(o1[:, m, :], p[:], Silu, bias=b_o1[:, m:m + 1])
        _mm_loop(nc, pp, w_o1, attn_in, 4, 8, 128, NE, o1_cb)
        out_sb = ap.tile([128, 4, NE], F32, tag="out32")
        def o2_cb(m, p):
            nc.vector.tensor_scalar_add(out_sb[:, m, :], p[:], b_o2[:, m:m + 1])
        _mm_loop(nc, pp, w_o2, o1, 8, 4, 128, NE, o2_cb)
        nc.sync.dma_start(D["out"][b].rearrange("m p e -> p m e"), out_sb[:])


_CACHE = {}


def _build():
    if "nc" in _CACHE:
        return _CACHE["nc"], _CACHE["names"]
    nc = bacc.Bacc("TRN2", target_bir_lowering=False, debug=False,
                   num_devices=N_CORES)
    D = {}

    def din(name, shape, dt):
        D[name] = nc.dram_tensor(name, shape, dt, kind="ExternalInput").ap()

    din("e_featT", [128, NE], F16)
    din("h_absT", [2, 128, BPC], F16)
    din("bias_n", [BPC, 128], F32)
    din("b_v3", [NHEADS, 64], F32)
    din("atomT", [4, 128, NE], F16)
    for nm, kt_n, dout in [
        ("w_q1e", 1, HIDDEN), ("w_q1h", 2, HIDDEN), ("w_q2", 8, HIDDEN),
        ("w_q3", 8, LATENT), ("w_k1", 4, HIDDEN), ("w_k2", 8, HIDDEN),
        ("w_k3", 8, LATENT), ("w_v1", 4, HIDDEN), ("w_v2", 8, HIDDEN),
        ("w_v3", 8, LATENT), ("w_o1", 4, HIDDEN), ("w_o2", 8, LATENT),
    ]:
        din(nm, [kt_n, 128, dout], F16)
    for nm, mt_n in [("b_q1", 8), ("b_q2", 8), ("b_q3", 4), ("b_k1", 8),
                     ("b_k2", 8), ("b_k3", 4), ("b_v1", 8), ("b_v2", 8),
                     ("b_o1", 8), ("b_o2", 4)]:
        din(nm, [mt_n, 128], F32)
    D["out"] = nc.dram_tensor("out", [BPC, 4, 128, NE], F32,
                              kind="ExternalOutput").ap()

    with tile.TileContext(nc) as tc, ExitStack() as ctx:
        _emit(nc, tc, ctx, D)
    nc.compile()
    names = [k for k in D if k != "out"]
    _CACHE["nc"] = nc
    _CACHE["names"] = names
    return nc, names


def _pad_w(W, kt_n):
    """[din, dout] fp32 -> [kt_n, 128, dout] fp16, K zero-padded."""
    din, dout = W.shape
    Wp = np.zeros((kt_n * 128, dout), np.float16)
    Wp[:din] = W.astype(np.float16)
    return Wp.reshape(kt_n, 128, dout)


def _prep_maps(h, z, pos, mask, e_feat, params, absorber_index):
    ai = int(absorber_index)
    h = np.asarray(h, np.float32)
    z = np.asarray(z)
    pos = np.asarray(pos, np.float32)
    mask = np.asarray(mask)
    e_feat = np.asarray(e_feat, np.float32)
    P = {k: [(np.asarray(W), np.asarray(bb)) for W, bb in v] if k != 'z_emb'
         else np.asarray(v) for k, v in params.items()}

    # geometry + static atom features (host: O(B*N) data prep)
    rel = pos - pos[:, ai, :][:, None, :]
    r = np.sqrt((rel * rel).sum(-1))
    u = rel / np.maximum(r, 1e-8)[..., None]
    valid = mask & (r <= CUTOFF)
    zr = P['z_emb'][z]
    rc = np.minimum(r, CUTOFF)
    centers = np.linspace(0.0, CUTOFF, RBF_DIM).astype(np.float32)
    delta = CUTOFF / (RBF_DIM - 1)
    gamma = 1.0 / (delta * delta + 1e-12)
    rr = np.exp(-gamma * (rc[..., None] - centers) ** 2)
    is_abs = np.zeros_like(r)
    is_abs[:, ai] = 1.0
    atom_static = np.concatenate(
        [h, zr, rr, u, is_abs[..., None]], axis=-1).astype(np.float32)  # [B,N,388]
    cut = 0.5 * (np.cos(np.pi * r / CUTOFF) + 1.0) * (r <= CUTOFF)
    radial = np.log(np.maximum(cut, 1e-8)).astype(np.float32)
    bias_n = np.where(valid, radial, np.float32(-1e9)).astype(np.float32)  # [B,N]

    # shared (per-core-identical) tensors
    shared = {}
    shared["e_featT"] = e_feat.T.astype(np.float16).copy()
    (Wq1, bq1), (Wq2, bq2), (Wq3, bq3) = P['q']
    shared["w_q1h"] = _pad_w(Wq1[:ATOM_DIM], 2)
    shared["w_q1e"] = _pad_w(Wq1[ATOM_DIM:], 1)
    shared["b_q1"] = bq1.astype(np.float32).reshape(8, 128)
    shared["w_q2"] = _pad_w(Wq2, 8)
    shared["b_q2"] = bq2.astype(np.float32).reshape(8, 128)
    shared["w_q3"] = _pad_w(Wq3 * np.float32(HEAD_DIM ** -0.5), 8)
    shared["b_q3"] = (bq3 * np.float32(HEAD_DIM ** -0.5)).astype(np.float32).reshape(4, 128)
    for nm, plist in [("k", P['k']), ("v", P['v'])]:
        (W1, b1), (W2, b2), (W3, b3) = plist
        shared[f"w_{nm}1"] = _pad_w(W1, 4)
        shared[f"b_{nm}1"] = b1.astype(np.float32).reshape(8, 128)
        shared[f"w_{nm}2"] = _pad_w(W2, 8)
        shared[f"b_{nm}2"] = b2.astype(np.float32).reshape(8, 128)
        shared[f"w_{nm}3"] = _pad_w(W3, 8)
        if nm == "k":
            shared["b_k3"] = b3.astype(np.float32).reshape(4, 128)
        else:
            shared["b_v3"] = b3.astype(np.float32).reshape(NHEADS, 64)
    (Wo1, bo1), (Wo2, bo2) = P['o']
    shared["w_o1"] = _pad_w(Wo1, 4)
    shared["b_o1"] = bo1.astype(np.float32).reshape(8, 128)
    shared["w_o2"] = _pad_w(Wo2, 8)
    shared["b_o2"] = bo2.astype(np.float32).reshape(4, 128)

    in_maps = []
    for c in range(N_CORES):
        bs = slice(c * BPC, (c + 1) * BPC)
        m = dict(shared)
        # atom_staticT: [feat(512 pad), cols=(b_local*128+n)] -> [4,128,512]
        a = atom_static[bs]                       # [4,128,388]
        aT = np.zeros((KSTAT, BPC * N), np.float16)
        aT[:ATOM_STATIC] = a.reshape(BPC * N, ATOM_STATIC).T
        m["atomT"] = aT.reshape(4, 128, BPC * N)
        m["h_absT"] = np.ascontiguousarray(
            h[bs, ai, :].T.astype(np.float16)).reshape(2, 128, BPC)
        m["bias_n"] = np.ascontiguousarray(bias_n[bs])  # [4,128]
        in_maps.append(m)
    return in_maps


class _Runner:
    """Compile once; run the NEFF on 8 cores repeatedly via PJRT shard_map."""

    def __init__(self):
        import jax
        from jax.sharding import Mesh, PartitionSpec
        from jax.experimental.shard_map import shard_map
        from concourse import bass2jax, mybir as _mybir

        nc, _ = _build()
        self.nc = nc
        bass2jax.install_neuronx_cc_hook()
        in_names, out_names, out_avals, zero_outs = [], [], [], []
        for alloc in nc.m.functions[0].allocations:
            if not isinstance(alloc, _mybir.MemoryLocationSet):
                continue
            name = alloc.memorylocations[0].name
            if alloc.kind == "ExternalInput":
                in_names.append(name)
            elif alloc.kind == "ExternalOutput":
                out_names.append(name)
                shape = tuple(alloc.tensor_shape)
                dtype = _mybir.dt.np(alloc.dtype)
                out_avals.append(jax.core.ShapedArray(shape, dtype))
                zero_outs.append(np.zeros(shape, dtype))
        partition_name = (nc.partition_id_tensor.name
                          if nc.partition_id_tensor else None)
        if partition_name is not None:
            in_names = [n for n in in_names if n != partition_name]
        self.in_names, self.out_names = in_names, out_names
        self.zero_outs = zero_outs
        n_params, n_outs = len(in_names), len(out_names)
        all_in_names = in_names + out_names
        if partition_name is not None:
            all_in_names = all_in_names + [partition_name]

        def _body(*args):
            operands = list(args)
            if partition_name is not None:
                operands.append(bass2jax.partition_id_tensor())
            outs = bass2jax._bass_exec_p.bind(
                *operands,
                out_avals=tuple(out_avals),
                in_names=tuple(all_in_names),
                out_names=tuple(out_names),
                lowering_input_output_aliases=(),
                sim_require_finite=True,
                sim_require_nnan=True,
                nc=nc,
            )
            return tuple(outs)

        devices = jax.devices()[:N_CORES]
        mesh = Mesh(np.asarray(devices), ("core",))
        self._fn = jax.jit(
            shard_map(_body, mesh=mesh,
                      in_specs=(PartitionSpec("core"),) * (n_params + n_outs),
                      out_specs=(PartitionSpec("core"),) * n_outs,
                      check_rep=False),
            donate_argnums=tuple(range(n_params, n_params + n_outs)),
            keep_unused=True)

    def concat_inputs(self, in_maps):
        return [np.concatenate([np.asarray(in_maps[c][n]) for c in range(N_CORES)],
                               axis=0) for n in self.in_names]

    def zeros(self):
        return [np.zeros((N_CORES * z.shape[0], *z.shape[1:]), z.dtype)
                for z in self.zero_outs]

    def run_raw(self, concat_in, zeros):
        return self._fn(*concat_in, *zeros)

    def run(self, in_maps):
        out_arrs = self._fn(*self.concat_inputs(in_maps), *self.zeros())
        out = {}
        for i, name in enumerate(self.out_names):
            a = np.asarray(out_arrs[i])
            out[name] = a.reshape(N_CORES, a.shape[0] // N_CORES, *a.shape[1:])
        return out


def _get_runner():
    if "runner" not in _CACHE:
        _CACHE["runner"] = _Runner()
    return _CACHE["runner"]


def kernel(h, z, pos, mask, e_feat, params, absorber_index):
    runner = _get_runner()
    in_maps = _prep_maps(h, z, pos, mask, e_feat, params, absorber_index)
    res = runner.run(in_maps)
    o = res["out"]                      # [cores, BPC, 4, 128, NE]
    o = o.reshape(B, LATENT, NE).transpose(0, 2, 1)   # [B, e, latent]
    return np.ascontiguousarray(o.astype(np.float32))


# revision 12
# speedup vs baseline: 184.2268x; 12.3335x over previous
"""Trainium2 Bass kernel for EnergyConditionedAtomAttention.

Sharding: data-parallel over B across 8 NeuronCores (4 batches/core).

Device dataflow (per core, feature-major activations xT[feat(part), rows(free)]):
  q-MLP L1 is decomposed: q_in = [h_abs | e_feat] row-concat, where the h_abs
  part is constant across the 512 energy rows of a batch. So
    q1_pre = W1e.T @ e_featT (shared over batches)  + per-batch (W1h.T @ h_abs + b1)
  and the per-batch term is a per-partition bias vector fused into the Silu ACT.
  All MLP layers run weights-stationary: out[dout_tile, rows] = W.T @ xT, which
  chains without transposes. v's last layer runs activation-stationary to get
  row-major v[n, hd] for the attention value matmul. Scores are computed
  directly transposed, sT[n, e] = kT_h.T @ qT_h, so softmax-exp feeds the
  attention matmul with no transpose; the denominator comes from a ones-matmul
  and is applied post-hoc (attn@v)/denom, with v's bias folded in after the
  divide (sum_n attn = 1).

All matmul operands fp16 (full PE rate), PSUM accumulation fp32, biases and
softmax chain fp32. Expected end-to-end rel error vs fp32 reference ~2e-4.
"""
import numpy as np
from contextlib import ExitStack

import concourse.bass as bass
import concourse.tile as tile
from concourse import bacc, mybir
from concourse import bass_utils

# ---- problem constants (hardcoded per contract) ----
B, N, NE = 32, 128, 512
ATOM_DIM, E_DIM, RBF_DIM, HIDDEN, LATENT = 256, 128, 64, 1024, 512
CUTOFF, MAX_Z, ZEMB, NHEADS = 5.0, 100, 64, 8
HEAD_DIM = LATENT // NHEADS  # 64
ATOM_STATIC = ATOM_DIM + ZEMB + RBF_DIM + 3 + 1  # 388
N_CORES = 8
BPC = B // N_CORES  # 4 batches per core

KSTAT = 512  # atom_static padded to 4 K-tiles
F16, F32 = mybir.dt.float16, mybir.dt.float32
Silu = mybir.ActivationFunctionType.Silu
Exp = mybir.ActivationFunctionType.Exp


def _mm_loop(nc, pp, wt, xt, kt_n, mt_n, dout_tile, rows, out_cb, tag="mlp"):
    """out[m] = sum_k  wt[:,k,m-slice].T @ xt[:,k,:rows];  out_cb(m, psum_ap)."""
    for m in range(mt_n):
        p = pp.tile([128, rows], F32, tag=tag)
        for k in range(kt_n):
            nc.tensor.matmul(
                p[:dout_tile, :],
                wt[:, k, m * dout_tile:(m + 1) * dout_tile],
                xt[:, k, 0:rows],
                start=(k == 0), stop=(k == kt_n - 1),
            )
        out_cb(m, p)


def _emit(nc, tc, ctx, D):
    sb = ctx.enter_context(tc.tile_pool(name="sb", bufs=1))
    wp = ctx.enter_context(tc.tile_pool(name="w", bufs=2))
    ap = ctx.enter_context(tc.tile_pool(name="acts", bufs=3))
    pq = ctx.enter_context(tc.tile_pool(name="persist", bufs=1))
    pp = ctx.enter_context(tc.tile_pool(name="ps", bufs=2, space="PSUM"))
    ph = ctx.enter_context(tc.tile_pool(name="psh", bufs=2, space="PSUM"))

    # ---- constants / small inputs ----
    e_feat = sb.tile([128, NE], F16, tag="e_feat")
    nc.sync.dma_start(e_feat[:], D["e_featT"][:])
    h_abs = sb.tile([128, 2, BPC], F16, tag="h_abs")
    nc.sync.dma_start(h_abs[:], D["h_absT"].rearrange("k p b -> p k b"))
    biasn = sb.tile([128, BPC], F32, tag="biasn")
    nc.sync.dma_start(biasn[:], D["bias_n"].rearrange("b p -> p b"))
    bv3 = sb.tile([64, NHEADS], F32, tag="bv3")
    nc.sync.dma_start(bv3[:], D["b_v3"].rearrange("h d -> d h"))
    ones = sb.tile([128, 64], F16, tag="ones")
    nc.vector.memset(ones[:], 1.0)

    def load_w(name, kt_n, dout, tag):
        t = wp.tile([128, kt_n, dout], F16, tag=tag)
        nc.sync.dma_start(t[:], D[name].rearrange("k p d -> p k d"))
        return t

    def load_b(name, mt_n):
        t = sb.tile([128, mt_n], F32, tag=name)
        nc.sync.dma_start(t[:], D[name].rearrange("m p -> p m"))
        return t

    w_q1e = load_w("w_q1e", 1, HIDDEN, "wq1e")
    w_q1h = load_w("w_q1h", 2, HIDDEN, "wq1h")
    b_q1 = load_b("b_q1", 8)
    w_q2 = load_w("w_q2", 8, HIDDEN, "w81")
    b_q2 = load_b("b_q2", 8)
    w_q3 = load_w("w_q3", 8, LATENT, "w85")
    b_q3 = load_b("b_q3", 4)

    # ---- q L1: shared e-part + per-batch h-part as bias ----
    # hterm[1024, b] = W1h.T @ h_absT + b_q1
    hterm = sb.tile([128, 8, BPC], F32, tag="hterm")
    for m in range(8):
        p = ph.tile([128, BPC], F32, tag="hterm")
        for k in range(2):
            nc.tensor.matmul(p[:], w_q1h[:, k, bass.ts(m, 128)], h_abs[:, k, :],
                             start=(k == 0), stop=(k == 1))
        nc.vector.tensor_scalar_add(hterm[:, m, :], p[:], b_q1[:, m:m + 1])
    # q1_pre[1024, 512] = W1e.T @ e_featT   (shared across batches)
    q1pre = sb.tile([128, 8, NE], F16, tag="q1pre")
    def q1pre_cb(m, p):
        nc.vector.tensor_copy(q1pre[:, m, :], p[:])
    _mm_loop(nc, pp, w_q1e, e_feat.rearrange("p (o n) -> p o n", o=1),
             1, 8, 128, NE, q1pre_cb)

    # ---- q L2/L3 per batch ----
    qT = []
    for b in range(BPC):
        q1 = ap.tile([128, 8, NE], F16, tag="a8")
        for m in range(8):
            nc.scalar.activation(q1[:, m, :], q1pre[:, m, :], Silu,
                                 bias=hterm[:, m, b:b + 1])
        q2 = ap.tile([128, 8, NE], F16, tag="a8")
        def q2_cb(m, p):
            nc.scalar.activation(q2[:, m, :], p[:], Silu, bias=b_q2[:, m:m + 1])
        _mm_loop(nc, pp, w_q2, q1, 8, 8, 128, NE, q2_cb)
        qb = pq.tile([128, 4, NE], F16, tag=f"qT{b}")
        def q3_cb(m, p):
            nc.vector.tensor_scalar_add(qb[:, m, :], p[:], b_q3[:, m:m + 1])
        _mm_loop(nc, pp, w_q3, q2, 8, 4, 128, NE, q3_cb)
        qT.append(qb)

    # ---- k/v MLPs on all 4 batches jointly (cols = b*128+n) ----
    atom = sb.tile([128, 4, NE], F16, tag="atom")
    nc.sync.dma_start(atom[:], D["atomT"].rearrange("k p c -> p k c"))

    w_k1 = load_w("w_k1", 4, HIDDEN, "w41")
    b_k1 = load_b("b_k1", 8)
    w_k2 = load_w("w_k2", 8, HIDDEN, "w81")
    b_k2 = load_b("b_k2", 8)
    w_k3 = load_w("w_k3", 8, LATENT, "w85")
    b_k3 = load_b("b_k3", 4)

    def mlp2(xt, w1, b1, w2, b2, kt1):
        y1 = ap.tile([128, 8, NE], F16, tag="a8")
        def l1_cb(m, p):
            nc.scalar.activation(y1[:, m, :], p[:], Silu, bias=b1[:, m:m + 1])
        _mm_loop(nc, pp, w1, xt, kt1, 8, 128, NE, l1_cb)
        y2 = ap.tile([128, 8, NE], F16, tag="a8")
        def l2_cb(m, p):
            nc.scalar.activation(y2[:, m, :], p[:], Silu, bias=b2[:, m:m + 1])
        _mm_loop(nc, pp, w2, y1, 8, 8, 128, NE, l2_cb)
        return y2

    k2t = mlp2(atom, w_k1, b_k1, w_k2, b_k2, 4)
    kT = pq.tile([128, 4, NE], F16, tag="kT")
    def k3_cb(m, p):
        nc.vector.tensor_scalar_add(kT[:, m, :], p[:], b_k3[:, m:m + 1])
    _mm_loop(nc, pp, w_k3, k2t, 8, 4, 128, NE, k3_cb)

    w_v1 = load_w("w_v1", 4, HIDDEN, "w41")
    b_v1 = load_b("b_v1", 8)
    w_v2 = load_w("w_v2", 8, HIDDEN, "w81")
    b_v2 = load_b("b_v2", 8)
    w_v3 = load_w("w_v3", 8, LATENT, "w85")

    v2t = mlp2(atom, w_v1, b_v1, w_v2, b_v2, 4)
    # v L3 activation-stationary -> row-major v[n, hd] per batch (bias folded in later)
    v_sb = pq.tile([128, BPC, LATENT], F16, tag="v_sb")
    for b in range(BPC):
        p = pp.tile([128, LATENT], F32, tag="mlp")
        for k in range(8):
            nc.tensor.matmul(p[:], v2t[:, k, bass.ts(b, 128)], w_v3[:, k, :],
                             start=(k == 0), stop=(k == 7))
        nc.vector.tensor_copy(v_sb[:, b, :], p[:])

    w_o1 = load_w("w_o1", 4, HIDDEN, "w41")
    b_o1 = load_b("b_o1", 8)
    w_o2 = load_w("w_o2", 8, LATENT, "w85")
    b_o2 = load_b("b_o2", 4)

    # ---- attention + o-MLP per batch ----
    for b in range(BPC):
        attn_in = ap.tile([128, 4, NE], F16, tag="attn_in")
        for h in range(NHEADS):
            pb, kt_i = 64 * (h % 2), h // 2
            # sT[n, e] = kT_h.T @ qT_h
            ps_s = pp.tile([128, NE], F32, tag="mlp")
            nc.tensor.matmul(ps_s[:], kT[pb:pb + 64, kt_i, bass.ts(b, 128)],
                             qT[b][pb:pb + 64, kt_i, :],
                             start=True, stop=True, tile_position=(pb, 0))
            # P = exp(s + bias_n)
            p_sb = ap.tile([128, NE], F16, tag="p")
            nc.scalar.activation(p_sb[:], ps_s[:], Exp, bias=biasn[:, b:b + 1])
            # att[hd, e] = v_h.T @ P ; den[e] = 1.T @ P (replicated on 64 parts)
            ps_a = ph.tile([64, NE], F32, tag="att")
            nc.tensor.matmul(ps_a[:], v_sb[:, b, bass.ts(h, 64)], p_sb[:],
                             start=True, stop=True)
            ps_d = ph.tile([64, NE], F32, tag="den")
            nc.tensor.matmul(ps_d[:], ones[:], p_sb[:], start=True, stop=True)
            rec = ap.tile([64, NE], F32, tag="rec")
            nc.vector.reciprocal(rec[:], ps_d[:])
            anorm = ap.tile([64, NE], F32, tag="anorm")
            nc.vector.tensor_tensor(anorm[:], ps_a[:], rec[:], mybir.AluOpType.mult)
            nc.vector.tensor_scalar_add(attn_in[pb:pb + 64, kt_i, :], anorm[:],
                                        bv3[:, h:h + 1])
        # o-MLP
        o1 = ap.tile([128, 8, NE], F16, tag="a8")
        def o1_cb(m, p):
            nc.scalar.activation(o1[:, m, :], p[:], Silu, bias=b_o1[:, m:m + 1])
        _mm_loop(nc, pp, w_o1, attn_in, 4, 8, 128, NE, o1_cb)
        out_sb = ap.tile([128, 4, NE], F32, tag="out32")
        def o2_cb(m, p):
            nc.vector.tensor_scalar_add(out_sb[:, m, :], p[:], b_o2[:, m:m + 1])
        _mm_loop(nc, pp, w_o2, o1, 8, 4, 128, NE, o2_cb)
        nc.sync.dma_start(D["out"][b].rearrange("m p e -> p m e"), out_sb[:])


_CACHE = {}


def _build():
    if "nc" in _CACHE:
        return _CACHE["nc"], _CACHE["names"]
    nc = bacc.Bacc("TRN2", target_bir_lowering=False, debug=False,
                   num_devices=N_CORES)
    D = {}

    def din(name, shape, dt):
        D[name] = nc.dram_tensor(name, shape, dt, kind="ExternalInput").ap()

    din("e_featT", [128, NE], F16)
    din("h_absT", [2, 128, BPC], F16)
    din("bias_n", [BPC, 128], F32)
    din("b_v3", [NHEADS, 64], F32)
    din("atomT", [4, 128, NE], F16)
    for nm, kt_n, dout in [
        ("w_q1e", 1, HIDDEN), ("w_q1h", 2, HIDDEN), ("w_q2", 8, HIDDEN),
        ("w_q3", 8, LATENT), ("w_k1", 4, HIDDEN), ("w_k2", 8, HIDDEN),
        ("w_k3", 8, LATENT), ("w_v1", 4, HIDDEN), ("w_v2", 8, HIDDEN),
        ("w_v3", 8, LATENT), ("w_o1", 4, HIDDEN), ("w_o2", 8, LATENT),
    ]:
        din(nm, [kt_n, 128, dout], F16)
    for nm, mt_n in [("b_q1", 8), ("b_q2", 8), ("b_q3", 4), ("b_k1", 8),
                     ("b_k2", 8), ("b_k3", 4), ("b_v1", 8), ("b_v2", 8),
                     ("b_o1", 8), ("b_o2", 4)]:
        din(nm, [mt_n, 128], F32)
    D["out"] = nc.dram_tensor("out", [BPC, 4, 128, NE], F32,
                              kind="ExternalOutput").ap()

    with tile.TileContext(nc) as tc, ExitStack() as ctx:
        _emit(nc, tc, ctx, D)
    nc.compile()
    names = [k for k in D if k != "out"]
    _CACHE["nc"] = nc
    _CACHE["names"] = names
    return nc, names


def _pad_w(W, kt_n):
    """[din, dout] fp32 -> [kt_n, 128, dout] fp16, K zero-padded."""
    din, dout = W.shape
    Wp = np.zeros((kt_n * 128, dout), np.float16)
    Wp[:din] = W.astype(np.float16)
    return Wp.reshape(kt_n, 128, dout)


def _prep_maps(h, z, pos, mask, e_feat, params, absorber_index):
    ai = int(absorber_index)
    h = np.asarray(h, np.float32)
    z = np.asarray(z)
    pos = np.asarray(pos, np.float32)
    mask = np.asarray(mask)
    e_feat = np.asarray(e_feat, np.float32)
    P = {k: [(np.asarray(W), np.asarray(bb)) for W, bb in v] if k != 'z_emb'
         else np.asarray(v) for k, v in params.items()}

    # geometry + static atom features (host: O(B*N) data prep)
    rel = pos - pos[:, ai, :][:, None, :]
    r = np.sqrt((rel * rel).sum(-1))
    u = rel / np.maximum(r, 1e-8)[..., None]
    valid = mask & (r <= CUTOFF)
    zr = P['z_emb'][z]
    rc = np.minimum(r, CUTOFF)
    centers = np.linspace(0.0, CUTOFF, RBF_DIM).astype(np.float32)
    delta = CUTOFF / (RBF_DIM - 1)
    gamma = 1.0 / (delta * delta + 1e-12)
    rr = np.exp(-gamma * (rc[..., None] - centers) ** 2)
    is_abs = np.zeros_like(r)
    is_abs[:, ai] = 1.0
    atom_static = np.concatenate(
        [h, zr, rr, u, is_abs[..., None]], axis=-1).astype(np.float32)  # [B,N,388]
    cut = 0.5 * (np.cos(np.pi * r / CUTOFF) + 1.0) * (r <= CUTOFF)
    radial = np.log(np.maximum(cut, 1e-8)).astype(np.float32)
    bias_n = np.where(valid, radial, np.float32(-1e9)).astype(np.float32)  # [B,N]

    # shared (per-core-identical) tensors
    shared = {}
    shared["e_featT"] = e_feat.T.astype(np.float16).copy()
    (Wq1, bq1), (Wq2, bq2), (Wq3, bq3) = P['q']
    shared["w_q1h"] = _pad_w(Wq1[:ATOM_DIM], 2)
    shared["w_q1e"] = _pad_w(Wq1[ATOM_DIM:], 1)
    shared["b_q1"] = bq1.astype(np.float32).reshape(8, 128)
    shared["w_q2"] = _pad_w(Wq2, 8)
    shared["b_q2"] = bq2.astype(np.float32).reshape(8, 128)
    shared["w_q3"] = _pad_w(Wq3 * np.float32(HEAD_DIM ** -0.5), 8)
    shared["b_q3"] = (bq3 * np.float32(HEAD_DIM ** -0.5)).astype(np.float32).reshape(4, 128)
    for nm, plist in [("k", P['k']), ("v", P['v'])]:
        (W1, b1), (W2, b2), (W3, b3) = plist
        shared[f"w_{nm}1"] = _pad_w(W1, 4)
        shared[f"b_{nm}1"] = b1.astype(np.float32).reshape(8, 128)
        shared[f"w_{nm}2"] = _pad_w(W2, 8)
        shared[f"b_{nm}2"] = b2.astype(np.float32).reshape(8, 128)
        shared[f"w_{nm}3"] = _pad_w(W3, 8)
        if nm == "k":
            shared["b_k3"] = b3.astype(np.float32).reshape(4, 128)
        else:
            shared["b_v3"] = b3.astype(np.float32).reshape(NHEADS, 64)
    (Wo1, bo1), (Wo2, bo2) = P['o']
    shared["w_o1"] = _pad_w(Wo1, 4)
    shared["b_o1"] = bo1.astype(np.float32).reshape(8, 128)
    shared["w_o2"] = _pad_w(Wo2, 8)
    shared["b_o2"] = bo2.astype(np.float32).reshape(4, 128)

    in_maps = []
    for c in range(N_CORES):
        bs = slice(c * BPC, (c + 1) * BPC)
        m = dict(shared)
        # atom_staticT: [feat(512 pad), cols=(b_local*128+n)] -> [4,128,512]
        a = atom_static[bs]                       # [4,128,388]
        aT = np.zeros((KSTAT, BPC * N), np.float16)
        aT[:ATOM_STATIC] = a.reshape(BPC * N, ATOM_STATIC).T
        m["atomT"] = aT.reshape(4, 128, BPC * N)
        m["h_absT"] = np.ascontiguousarray(
            h[bs, ai, :].T.astype(np.float16)).reshape(2, 128, BPC)
        m["bias_n"] = np.ascontiguousarray(bias_n[bs])  # [4,128]
        in_maps.append(m)
    return in_maps


class _Runner:
    """Compile once; run the NEFF on 8 cores repeatedly via PJRT shard_map."""

    def __init__(self):
        import jax
        from jax.sharding import Mesh, PartitionSpec
        from jax.experimental.shard_map import shard_map
        from concourse import bass2jax, mybir as _mybir

        nc, _ = _build()
        self.nc = nc
        bass2jax.install_neuronx_cc_hook()
        in_names, out_names, out_avals, zero_outs = [], [], [], []
        for alloc in nc.m.functions[0].allocations:
            if not isinstance(alloc, _mybir.MemoryLocationSet):
                continue
            name = alloc.memorylocations[0].name
            if alloc.kind == "ExternalInput":
                in_names.append(name)
            elif alloc.kind == "ExternalOutput":
                out_names.append(name)
                shape = tuple(alloc.tensor_shape)
                dtype = _mybir.dt.np(alloc.dtype)
                out_avals.append(jax.core.ShapedArray(shape, dtype))
                zero_outs.append(np.zeros(shape, dtype))
        partition_name = (nc.partition_id_tensor.name
                          if nc.partition_id_tensor else None)
        if partition_name is not None:
            in_names = [n for n in in_names if n != partition_name]
        self.in_names, self.out_names = in_names, out_names
        self.zero_outs = zero_outs
        n_params, n_outs = len(in_names), len(out_names)
        all_in_names = in_names + out_names
        if partition_name is not None:
            all_in_names = all_in_names + [partition_name]

        import jax.numpy as jnp
        from jax.sharding import NamedSharding

        def _body(*args):
            operands = list(args)
            if partition_name is not None:
                operands.append(bass2jax.partition_id_tensor())
            outs = bass2jax._bass_exec_p.bind(
                *operands,
                out_avals=tuple(out_avals),
                in_names=tuple(all_in_names),
                out_names=tuple(out_names),
                lowering_input_output_aliases=(),
                sim_require_finite=True,
                sim_require_nnan=True,
                nc=nc,
            )
            return tuple(outs)

        devices = jax.devices()[:N_CORES]
        mesh = Mesh(np.asarray(devices), ("core",))
        self._fn = jax.jit(
            shard_map(_body, mesh=mesh,
                      in_specs=(PartitionSpec("core"),) * (n_params + n_outs),
                      out_specs=(PartitionSpec("core"),) * n_outs,
                      check_rep=False),
            donate_argnums=tuple(range(n_params, n_params + n_outs)),
            keep_unused=True)
        zshard = NamedSharding(mesh, PartitionSpec("core"))
        self._zeros_fn = jax.jit(
            lambda: tuple(jnp.zeros((N_CORES * z.shape[0], *z.shape[1:]), z.dtype)
                          for z in zero_outs),
            out_shardings=tuple(zshard for _ in zero_outs))

    def concat_inputs(self, in_maps):
        return [np.concatenate([np.asarray(in_maps[c][n]) for c in range(N_CORES)],
                               axis=0) for n in self.in_names]

    def run_raw(self, concat_in):
        return self._fn(*concat_in, *self._zeros_fn())

    def run(self, in_maps):
        out_arrs = self.run_raw(self.concat_inputs(in_maps))
        out = {}
        for i, name in enumerate(self.out_names):
            a = np.asarray(out_arrs[i])
            out[name] = a.reshape(N_CORES, a.shape[0] // N_CORES, *a.shape[1:])
        return out


def _get_runner():
    if "runner" not in _CACHE:
        _CACHE["runner"] = _Runner()
    return _CACHE["runner"]


def kernel(h, z, pos, mask, e_feat, params, absorber_index):
    runner = _get_runner()
    in_maps = _prep_maps(h, z, pos, mask, e_feat, params, absorber_index)
    res = runner.run(in_maps)
    o = res["out"]                      # [cores, BPC, 4, 128, NE]
    o = o.reshape(B, LATENT, NE).transpose(0, 2, 1)   # [B, e, latent]
    return np.ascontiguousarray(o.astype(np.float32))


# revision 16
# speedup vs baseline: 3027.7428x; 16.4349x over previous
"""Trainium2 Bass kernel for EnergyConditionedAtomAttention.

Sharding: data-parallel over B across 8 NeuronCores (4 batches/core).

Device dataflow (per core, feature-major activations xT[feat(part), rows(free)]):
  q-MLP L1 is decomposed: q_in = [h_abs | e_feat] row-concat, where the h_abs
  part is constant across the 512 energy rows of a batch. So
    q1_pre = W1e.T @ e_featT (shared over batches)  + per-batch (W1h.T @ h_abs + b1)
  and the per-batch term is a per-partition bias vector fused into the Silu ACT.
  All MLP layers run weights-stationary: out[dout_tile, rows] = W.T @ xT, which
  chains without transposes. v's last layer runs activation-stationary to get
  row-major v[n, hd] for the attention value matmul. Scores are computed
  directly transposed, sT[n, e] = kT_h.T @ qT_h, so softmax-exp feeds the
  attention matmul with no transpose; the denominator comes from a ones-matmul
  and is applied post-hoc (attn@v)/denom, with v's bias folded in after the
  divide (sum_n attn = 1).

All matmul operands fp16 (full PE rate), PSUM accumulation fp32, biases and
softmax chain fp32. Expected end-to-end rel error vs fp32 reference ~2e-4.
"""
import numpy as np
from contextlib import ExitStack

import concourse.bass as bass
import concourse.tile as tile
from concourse import bacc, mybir
from concourse import bass_utils

# ---- problem constants (hardcoded per contract) ----
B, N, NE = 32, 128, 512
ATOM_DIM, E_DIM, RBF_DIM, HIDDEN, LATENT = 256, 128, 64, 1024, 512
CUTOFF, MAX_Z, ZEMB, NHEADS = 5.0, 100, 64, 8
HEAD_DIM = LATENT // NHEADS  # 64
ATOM_STATIC = ATOM_DIM + ZEMB + RBF_DIM + 3 + 1  # 388
N_CORES = 8
BPC = B // N_CORES  # 4 batches per core

KSTAT = 512  # atom_static padded to 4 K-tiles
F16, F32 = mybir.dt.float16, mybir.dt.float32
Silu = mybir.ActivationFunctionType.Silu
Exp = mybir.ActivationFunctionType.Exp


def _mm_loop(nc, pp, wt, xt, kt_n, mt_n, dout_tile, rows, out_cb, tag="mlp"):
    """out[m] = sum_k  wt[:,k,m-slice].T @ xt[:,k,:rows];  out_cb(m, psum_ap)."""
    for m in range(mt_n):
        p = pp.tile([128, rows], F32, tag=tag)
        for k in range(kt_n):
            nc.tensor.matmul(
                p[:dout_tile, :],
                wt[:, k, m * dout_tile:(m + 1) * dout_tile],
                xt[:, k, 0:rows],
                start=(k == 0), stop=(k == kt_n - 1),
            )
        out_cb(m, p)


def _emit(nc, tc, ctx, D, sfx=""):
    sb = ctx.enter_context(tc.tile_pool(name=f"sb{sfx}", bufs=1))
    wp = ctx.enter_context(tc.tile_pool(name=f"w{sfx}", bufs=2))
    ap = ctx.enter_context(tc.tile_pool(name=f"acts{sfx}", bufs=3))
    pq = ctx.enter_context(tc.tile_pool(name=f"persist{sfx}", bufs=1))
    pp = ctx.enter_context(tc.tile_pool(name=f"ps{sfx}", bufs=2, space="PSUM"))
    ph = ctx.enter_context(tc.tile_pool(name=f"psh{sfx}", bufs=2, space="PSUM"))

    # ---- constants / small inputs ----
    e_feat = sb.tile([128, NE], F16, tag="e_feat")
    nc.sync.dma_start(e_feat[:], D["e_featT"][:])
    h_abs = sb.tile([128, 2, BPC], F16, tag="h_abs")
    nc.sync.dma_start(h_abs[:], D["h_absT"].rearrange("k p b -> p k b"))
    biasn = sb.tile([128, BPC], F32, tag="biasn")
    nc.sync.dma_start(biasn[:], D["bias_n"].rearrange("b p -> p b"))
    bv3 = sb.tile([64, NHEADS], F32, tag="bv3")
    nc.sync.dma_start(bv3[:], D["b_v3"].rearrange("h d -> d h"))
    ones = sb.tile([128, 64], F16, tag="ones")
    nc.vector.memset(ones[:], 1.0)

    def load_w(name, kt_n, dout, tag):
        t = wp.tile([128, kt_n, dout], F16, tag=tag)
        nc.sync.dma_start(t[:], D[name].rearrange("k p d -> p k d"))
        return t

    def load_b(name, mt_n):
        t = sb.tile([128, mt_n], F32, tag=name)
        nc.sync.dma_start(t[:], D[name].rearrange("m p -> p m"))
        return t

    w_q1e = load_w("w_q1e", 1, HIDDEN, "wq1e")
    w_q1h = load_w("w_q1h", 2, HIDDEN, "wq1h")
    b_q1 = load_b("b_q1", 8)
    w_q2 = load_w("w_q2", 8, HIDDEN, "w81")
    b_q2 = load_b("b_q2", 8)
    w_q3 = load_w("w_q3", 8, LATENT, "w85")
    b_q3 = load_b("b_q3", 4)

    # ---- q L1: shared e-part + per-batch h-part as bias ----
    # hterm[1024, b] = W1h.T @ h_absT + b_q1
    hterm = sb.tile([128, 8, BPC], F32, tag="hterm")
    for m in range(8):
        p = ph.tile([128, BPC], F32, tag="hterm")
        for k in range(2):
            nc.tensor.matmul(p[:], w_q1h[:, k, bass.ts(m, 128)], h_abs[:, k, :],
                             start=(k == 0), stop=(k == 1))
        nc.vector.tensor_scalar_add(hterm[:, m, :], p[:], b_q1[:, m:m + 1])
    # q1_pre[1024, 512] = W1e.T @ e_featT   (shared across batches)
    q1pre = sb.tile([128, 8, NE], F16, tag="q1pre")
    def q1pre_cb(m, p):
        nc.vector.tensor_copy(q1pre[:, m, :], p[:])
    _mm_loop(nc, pp, w_q1e, e_feat.rearrange("p (o n) -> p o n", o=1),
             1, 8, 128, NE, q1pre_cb)

    # ---- q L2/L3 per batch ----
    qT = []
    for b in range(BPC):
        q1 = ap.tile([128, 8, NE], F16, tag="a8")
        for m in range(8):
            nc.scalar.activation(q1[:, m, :], q1pre[:, m, :], Silu,
                                 bias=hterm[:, m, b:b + 1])
        q2 = ap.tile([128, 8, NE], F16, tag="a8")
        def q2_cb(m, p):
            nc.scalar.activation(q2[:, m, :], p[:], Silu, bias=b_q2[:, m:m + 1])
        _mm_loop(nc, pp, w_q2, q1, 8, 8, 128, NE, q2_cb)
        qb = pq.tile([128, 4, NE], F16, tag=f"qT{b}")
        def q3_cb(m, p):
            nc.vector.tensor_scalar_add(qb[:, m, :], p[:], b_q3[:, m:m + 1])
        _mm_loop(nc, pp, w_q3, q2, 8, 4, 128, NE, q3_cb)
        qT.append(qb)

    # ---- k/v MLPs on all 4 batches jointly (cols = b*128+n) ----
    atom = sb.tile([128, 4, NE], F16, tag="atom")
    nc.sync.dma_start(atom[:], D["atomT"].rearrange("k p c -> p k c"))

    w_k1 = load_w("w_k1", 4, HIDDEN, "w41")
    b_k1 = load_b("b_k1", 8)
    w_k2 = load_w("w_k2", 8, HIDDEN, "w81")
    b_k2 = load_b("b_k2", 8)
    w_k3 = load_w("w_k3", 8, LATENT, "w85")
    b_k3 = load_b("b_k3", 4)

    def mlp2(xt, w1, b1, w2, b2, kt1):
        y1 = ap.tile([128, 8, NE], F16, tag="a8")
        def l1_cb(m, p):
            nc.scalar.activation(y1[:, m, :], p[:], Silu, bias=b1[:, m:m + 1])
        _mm_loop(nc, pp, w1, xt, kt1, 8, 128, NE, l1_cb)
        y2 = ap.tile([128, 8, NE], F16, tag="a8")
        def l2_cb(m, p):
            nc.scalar.activation(y2[:, m, :], p[:], Silu, bias=b2[:, m:m + 1])
        _mm_loop(nc, pp, w2, y1, 8, 8, 128, NE, l2_cb)
        return y2

    k2t = mlp2(atom, w_k1, b_k1, w_k2, b_k2, 4)
    kT = pq.tile([128, 4, NE], F16, tag="kT")
    def k3_cb(m, p):
        nc.vector.tensor_scalar_add(kT[:, m, :], p[:], b_k3[:, m:m + 1])
    _mm_loop(nc, pp, w_k3, k2t, 8, 4, 128, NE, k3_cb)

    w_v1 = load_w("w_v1", 4, HIDDEN, "w41")
    b_v1 = load_b("b_v1", 8)
    w_v2 = load_w("w_v2", 8, HIDDEN, "w81")
    b_v2 = load_b("b_v2", 8)
    w_v3 = load_w("w_v3", 8, LATENT, "w85")

    v2t = mlp2(atom, w_v1, b_v1, w_v2, b_v2, 4)
    # v L3 activation-stationary -> row-major v[n, hd] per batch (bias folded in later)
    v_sb = pq.tile([128, BPC, LATENT], F16, tag="v_sb")
    for b in range(BPC):
        p = pp.tile([128, LATENT], F32, tag="mlp")
        for k in range(8):
            nc.tensor.matmul(p[:], v2t[:, k, bass.ts(b, 128)], w_v3[:, k, :],
                             start=(k == 0), stop=(k == 7))
        nc.vector.tensor_copy(v_sb[:, b, :], p[:])

    w_o1 = load_w("w_o1", 4, HIDDEN, "w41")
    b_o1 = load_b("b_o1", 8)
    w_o2 = load_w("w_o2", 8, LATENT, "w85")
    b_o2 = load_b("b_o2", 4)

    # ---- attention + o-MLP per batch ----
    for b in range(BPC):
        attn_in = ap.tile([128, 4, NE], F16, tag="attn_in")
        for h in range(NHEADS):
            pb, kt_i = 64 * (h % 2), h // 2
            # sT[n, e] = kT_h.T @ qT_h
            ps_s = pp.tile([128, NE], F32, tag="mlp")
            nc.tensor.matmul(ps_s[:], kT[pb:pb + 64, kt_i, bass.ts(b, 128)],
                             qT[b][pb:pb + 64, kt_i, :],
                             start=True, stop=True, tile_position=(pb, 0))
            # P = exp(s + bias_n)
            p_sb = ap.tile([128, NE], F16, tag="p")
            nc.scalar.activation(p_sb[:], ps_s[:], Exp, bias=biasn[:, b:b + 1])
            # att[hd, e] = v_h.T @ P ; den[e] = 1.T @ P (replicated on 64 parts)
            ps_a = ph.tile([64, NE], F32, tag="att")
            nc.tensor.matmul(ps_a[:], v_sb[:, b, bass.ts(h, 64)], p_sb[:],
                             start=True, stop=True)
            ps_d = ph.tile([64, NE], F32, tag="den")
            nc.tensor.matmul(ps_d[:], ones[:], p_sb[:], start=True, stop=True)
            rec = ap.tile([64, NE], F32, tag="rec")
            nc.vector.reciprocal(rec[:], ps_d[:])
            anorm = ap.tile([64, NE], F32, tag="anorm")
            nc.vector.tensor_tensor(anorm[:], ps_a[:], rec[:], mybir.AluOpType.mult)
            nc.vector.tensor_scalar_add(attn_in[pb:pb + 64, kt_i, :], anorm[:],
                                        bv3[:, h:h + 1])
        # o-MLP
        o1 = ap.tile([128, 8, NE], F16, tag="a8")
        def o1_cb(m, p):
            nc.scalar.activation(o1[:, m, :], p[:], Silu, bias=b_o1[:, m:m + 1])
        _mm_loop(nc, pp, w_o1, attn_in, 4, 8, 128, NE, o1_cb)
        out_sb = ap.tile([128, 4, NE], F32, tag="out32")
        def o2_cb(m, p):
            nc.vector.tensor_scalar_add(out_sb[:, m, :], p[:], b_o2[:, m:m + 1])
        _mm_loop(nc, pp, w_o2, o1, 8, 4, 128, NE, o2_cb)
        nc.sync.dma_start(D["out"][b].rearrange("m p e -> p m e"), out_sb[:])


_CACHE = {}


def _build(repeats=1):
    key = f"nc{repeats}"
    if key in _CACHE:
        return _CACHE[key], _CACHE["names"]
    nc = bacc.Bacc("TRN2", target_bir_lowering=False, debug=False,
                   num_devices=N_CORES)
    D = {}

    def din(name, shape, dt):
        D[name] = nc.dram_tensor(name, shape, dt, kind="ExternalInput").ap()

    din("e_featT", [128, NE], F16)
    din("h_absT", [2, 128, BPC], F16)
    din("bias_n", [BPC, 128], F32)
    din("b_v3", [NHEADS, 64], F32)
    din("atomT", [4, 128, NE], F16)
    for nm, kt_n, dout in [
        ("w_q1e", 1, HIDDEN), ("w_q1h", 2, HIDDEN), ("w_q2", 8, HIDDEN),
        ("w_q3", 8, LATENT), ("w_k1", 4, HIDDEN), ("w_k2", 8, HIDDEN),
        ("w_k3", 8, LATENT), ("w_v1", 4, HIDDEN), ("w_v2", 8, HIDDEN),
        ("w_v3", 8, LATENT), ("w_o1", 4, HIDDEN), ("w_o2", 8, LATENT),
    ]:
        din(nm, [kt_n, 128, dout], F16)
    for nm, mt_n in [("b_q1", 8), ("b_q2", 8), ("b_q3", 4), ("b_k1", 8),
                     ("b_k2", 8), ("b_k3", 4), ("b_v1", 8), ("b_v2", 8),
                     ("b_o1", 8), ("b_o2", 4)]:
        din(nm, [mt_n, 128], F32)
    D["out"] = nc.dram_tensor("out", [BPC, 4, 128, NE], F32,
                              kind="ExternalOutput").ap()

    with tile.TileContext(nc) as tc:
        for rep in range(repeats):
            with ExitStack() as ctx:
                _emit(nc, tc, ctx, D, sfx=str(rep) if rep else "")
    nc.compile()
    names = [k for k in D if k != "out"]
    _CACHE[key] = nc
    _CACHE["names"] = names
    return nc, names


def _pad_w(W, kt_n):
    """[din, dout] fp32 -> [kt_n, 128, dout] fp16, K zero-padded."""
    din, dout = W.shape
    Wp = np.zeros((kt_n * 128, dout), np.float16)
    Wp[:din] = W.astype(np.float16)
    return Wp.reshape(kt_n, 128, dout)


def _prep_maps(h, z, pos, mask, e_feat, params, absorber_index):
    ai = int(absorber_index)
    h = np.asarray(h, np.float32)
    z = np.asarray(z)
    pos = np.asarray(pos, np.float32)
    mask = np.asarray(mask)
    e_feat = np.asarray(e_feat, np.float32)
    P = {k: [(np.asarray(W), np.asarray(bb)) for W, bb in v] if k != 'z_emb'
         else np.asarray(v) for k, v in params.items()}

    # geometry + static atom features (host: O(B*N) data prep)
    rel = pos - pos[:, ai, :][:, None, :]
    r = np.sqrt((rel * rel).sum(-1))
    u = rel / np.maximum(r, 1e-8)[..., None]
    valid = mask & (r <= CUTOFF)
    zr = P['z_emb'][z]
    rc = np.minimum(r, CUTOFF)
    centers = np.linspace(0.0, CUTOFF, RBF_DIM).astype(np.float32)
    delta = CUTOFF / (RBF_DIM - 1)
    gamma = 1.0 / (delta * delta + 1e-12)
    rr = np.exp(-gamma * (rc[..., None] - centers) ** 2)
    is_abs = np.zeros_like(r)
    is_abs[:, ai] = 1.0
    atom_static = np.concatenate(
        [h, zr, rr, u, is_abs[..., None]], axis=-1).astype(np.float32)  # [B,N,388]
    cut = 0.5 * (np.cos(np.pi * r / CUTOFF) + 1.0) * (r <= CUTOFF)
    radial = np.log(np.maximum(cut, 1e-8)).astype(np.float32)
    bias_n = np.where(valid, radial, np.float32(-1e9)).astype(np.float32)  # [B,N]

    # shared (per-core-identical) tensors
    shared = {}
    shared["e_featT"] = e_feat.T.astype(np.float16).copy()
    (Wq1, bq1), (Wq2, bq2), (Wq3, bq3) = P['q']
    shared["w_q1h"] = _pad_w(Wq1[:ATOM_DIM], 2)
    shared["w_q1e"] = _pad_w(Wq1[ATOM_DIM:], 1)
    shared["b_q1"] = bq1.astype(np.float32).reshape(8, 128)
    shared["w_q2"] = _pad_w(Wq2, 8)
    shared["b_q2"] = bq2.astype(np.float32).reshape(8, 128)
    shared["w_q3"] = _pad_w(Wq3 * np.float32(HEAD_DIM ** -0.5), 8)
    shared["b_q3"] = (bq3 * np.float32(HEAD_DIM ** -0.5)).astype(np.float32).reshape(4, 128)
    for nm, plist in [("k", P['k']), ("v", P['v'])]:
        (W1, b1), (W2, b2), (W3, b3) = plist
        shared[f"w_{nm}1"] = _pad_w(W1, 4)
        shared[f"b_{nm}1"] = b1.astype(np.float32).reshape(8, 128)
        shared[f"w_{nm}2"] = _pad_w(W2, 8)
        shared[f"b_{nm}2"] = b2.astype(np.float32).reshape(8, 128)
        shared[f"w_{nm}3"] = _pad_w(W3, 8)
        if nm == "k":
            shared["b_k3"] = b3.astype(np.float32).reshape(4, 128)
        else:
            shared["b_v3"] = b3.astype(np.float32).reshape(NHEADS, 64)
    (Wo1, bo1), (Wo2, bo2) = P['o']
    shared["w_o1"] = _pad_w(Wo1, 4)
    shared["b_o1"] = bo1.astype(np.float32).reshape(8, 128)
    shared["w_o2"] = _pad_w(Wo2, 8)
    shared["b_o2"] = bo2.astype(np.float32).reshape(4, 128)

    in_maps = []
    for c in range(N_CORES):
        bs = slice(c * BPC, (c + 1) * BPC)
        m = dict(shared)
        # atom_staticT: [feat(512 pad), cols=(b_local*128+n)] -> [4,128,512]
        a = atom_static[bs]                       # [4,128,388]
        aT = np.zeros((KSTAT, BPC * N), np.float16)
        aT[:ATOM_STATIC] = a.reshape(BPC * N, ATOM_STATIC).T
        m["atomT"] = aT.reshape(4, 128, BPC * N)
        m["h_absT"] = np.ascontiguousarray(
            h[bs, ai, :].T.astype(np.float16)).reshape(2, 128, BPC)
        m["bias_n"] = np.ascontiguousarray(bias_n[bs])  # [4,128]
        in_maps.append(m)
    return in_maps


class _Runner:
    """Compile once; run the NEFF on 8 cores repeatedly via PJRT shard_map."""

    def __init__(self, repeats=1):
        import jax
        from jax.sharding import Mesh, PartitionSpec
        from jax.experimental.shard_map import shard_map
        from concourse import bass2jax, mybir as _mybir

        nc, _ = _build(repeats)
        self.nc = nc
        bass2jax.install_neuronx_cc_hook()
        in_names, out_names, out_avals, zero_outs = [], [], [], []
        for alloc in nc.m.functions[0].allocations:
            if not isinstance(alloc, _mybir.MemoryLocationSet):
                continue
            name = alloc.memorylocations[0].name
            if alloc.kind == "ExternalInput":
                in_names.append(name)
            elif alloc.kind == "ExternalOutput":
                out_names.append(name)
                shape = tuple(alloc.tensor_shape)
                dtype = _mybir.dt.np(alloc.dtype)
                out_avals.append(jax.core.ShapedArray(shape, dtype))
                zero_outs.append(np.zeros(shape, dtype))
        partition_name = (nc.partition_id_tensor.name
                          if nc.partition_id_tensor else None)
        if partition_name is not None:
            in_names = [n for n in in_names if n != partition_name]
        self.in_names, self.out_names = in_names, out_names
        self.zero_outs = zero_outs
        n_params, n_outs = len(in_names), len(out_names)
        all_in_names = in_names + out_names
        if partition_name is not None:
            all_in_names = all_in_names + [partition_name]

        import jax.numpy as jnp
        from jax.sharding import NamedSharding

        def _body(*args):
            operands = list(args)
            if partition_name is not None:
                operands.append(bass2jax.partition_id_tensor())
            outs = bass2jax._bass_exec_p.bind(
                *operands,
                out_avals=tuple(out_avals),
                in_names=tuple(all_in_names),
                out_names=tuple(out_names),
                lowering_input_output_aliases=(),
                sim_require_finite=True,
                sim_require_nnan=True,
                nc=nc,
            )
            return tuple(outs)

        devices = jax.devices()[:N_CORES]
        mesh = Mesh(np.asarray(devices), ("core",))
        self._fn = jax.jit(
            shard_map(_body, mesh=mesh,
                      in_specs=(PartitionSpec("core"),) * (n_params + n_outs),
                      out_specs=(PartitionSpec("core"),) * n_outs,
                      check_rep=False),
            donate_argnums=tuple(range(n_params, n_params + n_outs)),
            keep_unused=True)
        zshard = NamedSharding(mesh, PartitionSpec("core"))
        self._zeros_fn = jax.jit(
            lambda: tuple(jnp.zeros((N_CORES * z.shape[0], *z.shape[1:]), z.dtype)
                          for z in zero_outs),
            out_shardings=tuple(zshard for _ in zero_outs))

    def concat_inputs(self, in_maps):
        return [np.concatenate([np.asarray(in_maps[c][n]) for c in range(N_CORES)],
                               axis=0) for n in self.in_names]

    def run_raw(self, concat_in):
        return self._fn(*concat_in, *self._zeros_fn())

    def run(self, in_maps):
        out_arrs = self.run_raw(self.concat_inputs(in_maps))
        out = {}
        for i, name in enumerate(self.out_names):
            a = np.asarray(out_arrs[i])
            out[name] = a.reshape(N_CORES, a.shape[0] // N_CORES, *a.shape[1:])
        return out


def _get_runner(repeats=1):
    key = f"runner{repeats}"
    if key not in _CACHE:
        _CACHE[key] = _Runner(repeats)
    return _CACHE[key]


def kernel(h, z, pos, mask, e_feat, params, absorber_index):
    runner = _get_runner()
    in_maps = _prep_maps(h, z, pos, mask, e_feat, params, absorber_index)
    res = runner.run(in_maps)
    o = res["out"]                      # [cores, BPC, 4, 128, NE]
    o = o.reshape(B, LATENT, NE).transpose(0, 2, 1)   # [B, e, latent]
    return np.ascontiguousarray(o.astype(np.float32))


# revision 23
# speedup vs baseline: 236833.4890x; 78.2211x over previous
"""Trainium2 Bass kernel for EnergyConditionedAtomAttention.

Sharding: data-parallel over B across 8 NeuronCores (4 batches/core).

Device dataflow (per core, feature-major activations xT[feat(part), rows(free)]):
  q-MLP L1 is decomposed: q_in = [h_abs | e_feat] row-concat, where the h_abs
  part is constant across the 512 energy rows of a batch. So
    q1_pre = W1e.T @ e_featT (shared over batches)  + per-batch (W1h.T @ h_abs + b1)
  and the per-batch term is a per-partition bias vector fused into the Silu ACT.
  All MLP layers run weights-stationary: out[dout_tile, rows] = W.T @ xT, which
  chains without transposes. v's last layer runs activation-stationary to get
  row-major v[n, hd] for the attention value matmul. Scores are computed
  directly transposed, sT[n, e] = kT_h.T @ qT_h, so softmax-exp feeds the
  attention matmul with no transpose; the denominator comes from a ones-matmul
  and is applied post-hoc (attn@v)/denom, with v's bias folded in after the
  divide (sum_n attn = 1).

All matmul operands fp16 (full PE rate), PSUM accumulation fp32, biases and
softmax chain fp32. Expected end-to-end rel error vs fp32 reference ~2e-4.
"""
import numpy as np
from contextlib import ExitStack

import concourse.bass as bass
import concourse.tile as tile
from concourse import bacc, mybir
from concourse import bass_utils

# ---- problem constants (hardcoded per contract) ----
B, N, NE = 32, 128, 512
ATOM_DIM, E_DIM, RBF_DIM, HIDDEN, LATENT = 256, 128, 64, 1024, 512
CUTOFF, MAX_Z, ZEMB, NHEADS = 5.0, 100, 64, 8
HEAD_DIM = LATENT // NHEADS  # 64
ATOM_STATIC = ATOM_DIM + ZEMB + RBF_DIM + 3 + 1  # 388
N_CORES = 8
BPC = B // N_CORES  # 4 batches per core

KSTAT = 512  # atom_static padded to 4 K-tiles
F16, F32 = mybir.dt.float16, mybir.dt.float32
Silu = mybir.ActivationFunctionType.Silu
Exp = mybir.ActivationFunctionType.Exp


def _mm_loop(nc, pp, wt, xt, kt_n, mt_n, dout_tile, rows, out_cb, tag="mlp"):
    """out[m] = sum_k  wt[:,k,m-slice].T @ xt[:,k,:rows];  out_cb(m, psum_ap)."""
    for m in range(mt_n):
        p = pp.tile([128, rows], F32, tag=tag)
        for k in range(kt_n):
            nc.tensor.matmul(
                p[:dout_tile, :],
                wt[:, k, m * dout_tile:(m + 1) * dout_tile],
                xt[:, k, 0:rows],
                start=(k == 0), stop=(k == kt_n - 1),
            )
        out_cb(m, p)


def _emit_weights(nc, tc, ctx, D):
    """One-time loads: all weights + constants resident in SBUF."""
    sb = ctx.enter_context(tc.tile_pool(name="sbw", bufs=1))
    W = {}
    e_feat = sb.tile([128, NE], F16, tag="e_feat")
    nc.sync.dma_start(e_feat[:], D["e_featT"][:])
    W["e_feat"] = e_feat
    bv3 = sb.tile([64, NHEADS], F32, tag="bv3")
    nc.sync.dma_start(bv3[:], D["b_v3"].rearrange("h d -> d h"))
    W["bv3"] = bv3
    ones = sb.tile([128, 64], F16, tag="ones")
    nc.vector.memset(ones[:], 1.0)
    W["ones"] = ones

    def load_w(name, kt_n, dout):
        t = sb.tile([128, kt_n, dout], F16, tag=name)
        nc.sync.dma_start(t[:], D[name].rearrange("k p d -> p k d"))
        W[name] = t

    def load_b(name, mt_n):
        t = sb.tile([128, mt_n], F32, tag=name)
        nc.sync.dma_start(t[:], D[name].rearrange("m p -> p m"))
        W[name] = t

    load_w("w_q1e", 1, HIDDEN); load_w("w_q1h", 2, HIDDEN); load_b("b_q1", 8)
    load_w("w_k1", 4, HIDDEN); load_b("b_k1", 8)
    load_w("w_q2", 8, HIDDEN); load_b("b_q2", 8)
    load_w("w_q3", 8, LATENT); load_b("b_q3", 4)
    load_w("w_k2", 8, HIDDEN); load_b("b_k2", 8)
    load_w("w_k3", 8, LATENT); load_b("b_k3", 4)
    load_w("w_v1", 4, HIDDEN); load_b("b_v1", 8)
    load_w("w_v2", 8, HIDDEN); load_b("b_v2", 8)
    load_w("w_v3", 8, LATENT)
    load_w("w_o1", 4, HIDDEN); load_b("b_o1", 8)
    load_w("w_o2", 8, LATENT); load_b("b_o2", 4)
    return W


def _emit_compute(nc, tc, ctx, D, W, sfx=""):
    sb = ctx.enter_context(tc.tile_pool(name=f"sb{sfx}", bufs=1))
    ap = ctx.enter_context(tc.tile_pool(name=f"acts{sfx}", bufs=3))
    pq = ctx.enter_context(tc.tile_pool(name=f"persist{sfx}", bufs=1))
    pp = ctx.enter_context(tc.tile_pool(name=f"ps{sfx}", bufs=2, space="PSUM"))
    ph = ctx.enter_context(tc.tile_pool(name=f"psh{sfx}", bufs=2, space="PSUM"))

    e_feat, ones, bv3 = W["e_feat"], W["ones"], W["bv3"]
    w_q1e, w_q1h, b_q1 = W["w_q1e"], W["w_q1h"], W["b_q1"]
    w_q2, b_q2, w_q3, b_q3 = W["w_q2"], W["b_q2"], W["w_q3"], W["b_q3"]
    w_k1, b_k1, w_k2, b_k2 = W["w_k1"], W["b_k1"], W["w_k2"], W["b_k2"]
    w_k3, b_k3 = W["w_k3"], W["b_k3"]
    w_v1, b_v1, w_v2, b_v2 = W["w_v1"], W["b_v1"], W["w_v2"], W["b_v2"]
    w_v3 = W["w_v3"]
    w_o1, b_o1, w_o2, b_o2 = W["w_o1"], W["b_o1"], W["w_o2"], W["b_o2"]

    # ---- per-invocation inputs (SWDGE: don't queue behind the weight stream) ----
    h_abs = sb.tile([128, 2, BPC], F16, tag="h_abs")
    nc.gpsimd.dma_start(h_abs[:], D["h_absT"].rearrange("k p b -> p k b"))
    biasn = sb.tile([128, BPC], F32, tag="biasn")
    nc.gpsimd.dma_start(biasn[:], D["bias_n"].rearrange("b p -> p b"))

    # ---- q L1: shared e-part + per-batch h-part as bias ----
    # hterm[1024, b] = W1h.T @ h_absT + b_q1
    hterm = sb.tile([128, 8, BPC], F32, tag="hterm")
    for m in range(8):
        p = ph.tile([128, BPC], F32, tag="hterm")
        for k in range(2):
            nc.tensor.matmul(p[:], w_q1h[:, k, bass.ts(m, 128)], h_abs[:, k, :],
                             start=(k == 0), stop=(k == 1))
        nc.vector.tensor_scalar_add(hterm[:, m, :], p[:], b_q1[:, m:m + 1])
    # q1_pre[1024, 512] = W1e.T @ e_featT   (shared across batches)
    q1pre = sb.tile([128, 8, NE], F16, tag="q1pre")
    def q1pre_cb(m, p):
        nc.vector.tensor_copy(q1pre[:, m, :], p[:])
    _mm_loop(nc, pp, w_q1e, e_feat.rearrange("p (o n) -> p o n", o=1),
             1, 8, 128, NE, q1pre_cb)

    # ---- k L1 early: fills PE while the q1 silu chain drains ----
    atom = sb.tile([128, 4, NE], F16, tag="atom")
    nc.gpsimd.dma_start(atom[:], D["atomT"].rearrange("k p c -> p k c"))
    k1t = ap.tile([128, 8, NE], F16, tag="a8")
    def k1_cb(m, p):
        nc.scalar.activation(k1t[:, m, :], p[:], Silu, bias=b_k1[:, m:m + 1])
    _mm_loop(nc, pp, w_k1, atom, 4, 8, 128, NE, k1_cb)

    # ---- q L2/L3 per batch ----
    qT = []
    for b in range(BPC):
        q1 = ap.tile([128, 8, NE], F16, tag="a8")
        for m in range(8):
            nc.scalar.activation(q1[:, m, :], q1pre[:, m, :], Silu,
                                 bias=hterm[:, m, b:b + 1])
        q2 = ap.tile([128, 8, NE], F16, tag="a8")
        def q2_cb(m, p):
            nc.scalar.activation(q2[:, m, :], p[:], Silu, bias=b_q2[:, m:m + 1])
        _mm_loop(nc, pp, w_q2, q1, 8, 8, 128, NE, q2_cb)
        qb = pq.tile([128, 4, NE], F16, tag=f"qT{b}")
        def q3_cb(m, p):
            nc.vector.tensor_scalar_add(qb[:, m, :], p[:], b_q3[:, m:m + 1])
        _mm_loop(nc, pp, w_q3, q2, 8, 4, 128, NE, q3_cb)
        qT.append(qb)

    # ---- k/v MLPs on all 4 batches jointly (cols = b*128+n) ----
    def mlp2(xt, w1, b1, w2, b2, kt1):
        y1 = ap.tile([128, 8, NE], F16, tag="a8")
        def l1_cb(m, p):
            nc.scalar.activation(y1[:, m, :], p[:], Silu, bias=b1[:, m:m + 1])
        _mm_loop(nc, pp, w1, xt, kt1, 8, 128, NE, l1_cb)
        y2 = ap.tile([128, 8, NE], F16, tag="a8")
        def l2_cb(m, p):
            nc.scalar.activation(y2[:, m, :], p[:], Silu, bias=b2[:, m:m + 1])
        _mm_loop(nc, pp, w2, y1, 8, 8, 128, NE, l2_cb)
        return y2

    k2t = ap.tile([128, 8, NE], F16, tag="a8")
    def k2_cb(m, p):
        nc.scalar.activation(k2t[:, m, :], p[:], Silu, bias=b_k2[:, m:m + 1])
    _mm_loop(nc, pp, w_k2, k1t, 8, 8, 128, NE, k2_cb)
    kT = pq.tile([128, 4, NE], F16, tag="kT")
    def k3_cb(m, p):
        nc.vector.tensor_scalar_add(kT[:, m, :], p[:], b_k3[:, m:m + 1])
    _mm_loop(nc, pp, w_k3, k2t, 8, 4, 128, NE, k3_cb)

    v2t = mlp2(atom, w_v1, b_v1, w_v2, b_v2, 4)
    # v L3 activation-stationary -> row-major v[n, hd], interleaved with ones
    # blocks: v_aug[:, b, 128h:128h+64] = v_h, [128h+64:128h+128] = 1.0, so the
    # attention matmul lhsT=[v_h | 1] yields att on partitions 0-63 and the
    # softmax denominator replicated on 64-127 in a single matmul.
    v_aug = pq.tile([128, BPC, 2 * LATENT], F16, tag="v_aug")
    nc.vector.memset(
        v_aug[:].rearrange("p b (h t d) -> p b h t d", h=NHEADS, t=2)[:, :, :, 1, :],
        1.0)

    def v3_phase(b):
        p = pp.tile([128, LATENT], F32, tag="mlp")
        for k in range(8):
            nc.tensor.matmul(p[:], v2t[:, k, bass.ts(b, 128)], w_v3[:, k, :],
                             start=(k == 0), stop=(k == 7))
        va = v_aug[:, b, :].rearrange("p (h t d) -> p h t d", h=NHEADS, t=2)
        nc.vector.tensor_copy(va[:, :, 0, :],
                              p[:].rearrange("p (h d) -> p h d", h=NHEADS))

    # ---- attention + o-MLP, software-pipelined across batches ----
    def attn_phase(b):
        attn_in = ap.tile([128, 4, NE], F16, tag="attn_in", bufs=2)
        for h in range(NHEADS):
            pb, kt_i = 64 * (h % 2), h // 2
            # sT[n, e] = kT_h.T @ qT_h
            ps_s = pp.tile([128, NE], F32, tag="mlp")
            nc.tensor.matmul(ps_s[:], kT[pb:pb + 64, kt_i, bass.ts(b, 128)],
                             qT[b][pb:pb + 64, kt_i, :],
                             start=True, stop=True, tile_position=(pb, 0))
            # P = exp(s + bias_n)
            p_sb = ap.tile([128, NE], F16, tag="p")
            nc.scalar.activation(p_sb[:], ps_s[:], Exp, bias=biasn[:, b:b + 1])
            # one matmul: att[hd, e] on partitions 0-63, denom replicated on 64-127
            ps_ad = ph.tile([128, NE], F32, tag="attps", bufs=3)
            nc.tensor.matmul(ps_ad[:], v_aug[:, b, bass.ts(h, 128)], p_sb[:],
                             start=True, stop=True)
            rec = ap.tile([64, NE], F32, tag="rec", bufs=2)
            nc.vector.reciprocal(rec[:], ps_ad[64:128, :])
            anorm = ap.tile([64, NE], F32, tag="anorm", bufs=2)
            nc.vector.tensor_tensor(anorm[:], ps_ad[0:64, :], rec[:],
                                    mybir.AluOpType.mult)
            nc.vector.tensor_scalar_add(attn_in[pb:pb + 64, kt_i, :], anorm[:],
                                        bv3[:, h:h + 1])
        return attn_in

    def o_phase(b, attn_in):
        o1 = ap.tile([128, 8, NE], F16, tag="a8")
        def o1_cb(m, p):
            nc.scalar.activation(o1[:, m, :], p[:], Silu, bias=b_o1[:, m:m + 1])
        _mm_loop(nc, pp, w_o1, attn_in, 4, 8, 128, NE, o1_cb)
        def o2_cb(m, p):
            ot = ap.tile([128, NE], F32, tag="out32", bufs=3)
            nc.vector.tensor_scalar_add(ot[:], p[:], b_o2[:, m:m + 1])
            nc.sync.dma_start(D["out"][b, m], ot[:])
        _mm_loop(nc, pp, w_o2, o1, 8, 4, 128, NE, o2_cb)

    # pipeline: v3 one batch ahead of attn; attn one batch ahead of o-MLP
    v3_phase(0)
    v3_phase(1)
    pend = {0: attn_phase(0)}
    for b in range(1, BPC):
        if b + 1 < BPC:
            v3_phase(b + 1)
        pend[b] = attn_phase(b)
        o_phase(b - 1, pend.pop(b - 1))
    o_phase(BPC - 1, pend.pop(BPC - 1))


_CACHE = {}


def _build(repeats=1):
    key = f"nc{repeats}"
    if key in _CACHE:
        return _CACHE[key], _CACHE["names"]
    nc = bacc.Bacc("TRN2", target_bir_lowering=False, debug=False,
                   num_devices=N_CORES)
    D = {}

    def din(name, shape, dt):
        D[name] = nc.dram_tensor(name, shape, dt, kind="ExternalInput").ap()

    din("e_featT", [128, NE], F16)
    din("h_absT", [2, 128, BPC], F16)
    din("bias_n", [BPC, 128], F32)
    din("b_v3", [NHEADS, 64], F32)
    din("atomT", [4, 128, NE], F16)
    for nm, kt_n, dout in [
        ("w_q1e", 1, HIDDEN), ("w_q1h", 2, HIDDEN), ("w_q2", 8, HIDDEN),
        ("w_q3", 8, LATENT), ("w_k1", 4, HIDDEN), ("w_k2", 8, HIDDEN),
        ("w_k3", 8, LATENT), ("w_v1", 4, HIDDEN), ("w_v2", 8, HIDDEN),
        ("w_v3", 8, LATENT), ("w_o1", 4, HIDDEN), ("w_o2", 8, LATENT),
    ]:
        din(nm, [kt_n, 128, dout], F16)
    for nm, mt_n in [("b_q1", 8), ("b_q2", 8), ("b_q3", 4), ("b_k1", 8),
                     ("b_k2", 8), ("b_k3", 4), ("b_v1", 8), ("b_v2", 8),
                     ("b_o1", 8), ("b_o2", 4)]:
        din(nm, [mt_n, 128], F32)
    D["out"] = nc.dram_tensor("out", [BPC, 4, 128, NE], F32,
                              kind="ExternalOutput").ap()

    with tile.TileContext(nc) as tc, ExitStack() as wctx:
        W = _emit_weights(nc, tc, wctx, D)
        for rep in range(repeats):
            with ExitStack() as ctx:
                _emit_compute(nc, tc, ctx, D, W, sfx=str(rep) if rep else "")
    nc.compile()
    names = [k for k in D if k != "out"]
    _CACHE[key] = nc
    _CACHE["names"] = names
    return nc, names


def _pad_w(W, kt_n):
    """[din, dout] fp32 -> [kt_n, 128, dout] fp16, K zero-padded."""
    din, dout = W.shape
    Wp = np.zeros((kt_n * 128, dout), np.float16)
    Wp[:din] = W.astype(np.float16)
    return Wp.reshape(kt_n, 128, dout)


def _prep_maps(h, z, pos, mask, e_feat, params, absorber_index):
    ai = int(absorber_index)
    h = np.asarray(h, np.float32)
    z = np.asarray(z)
    pos = np.asarray(pos, np.float32)
    mask = np.asarray(mask)
    e_feat = np.asarray(e_feat, np.float32)
    P = {k: [(np.asarray(W), np.asarray(bb)) for W, bb in v] if k != 'z_emb'
         else np.asarray(v) for k, v in params.items()}

    # geometry + static atom features (host: O(B*N) data prep)
    rel = pos - pos[:, ai, :][:, None, :]
    r = np.sqrt((rel * rel).sum(-1))
    u = rel / np.maximum(r, 1e-8)[..., None]
    valid = mask & (r <= CUTOFF)
    zr = P['z_emb'][z]
    rc = np.minimum(r, CUTOFF)
    centers = np.linspace(0.0, CUTOFF, RBF_DIM).astype(np.float32)
    delta = CUTOFF / (RBF_DIM - 1)
    gamma = 1.0 / (delta * delta + 1e-12)
    rr = np.exp(-gamma * (rc[..., None] - centers) ** 2)
    is_abs = np.zeros_like(r)
    is_abs[:, ai] = 1.0
    atom_static = np.concatenate(
        [h, zr, rr, u, is_abs[..., None]], axis=-1).astype(np.float32)  # [B,N,388]
    cut = 0.5 * (np.cos(np.pi * r / CUTOFF) + 1.0) * (r <= CUTOFF)
    radial = np.log(np.maximum(cut, 1e-8)).astype(np.float32)
    bias_n = np.where(valid, radial, np.float32(-1e9)).astype(np.float32)  # [B,N]

    # shared (per-core-identical) tensors
    shared = {}
    shared["e_featT"] = e_feat.T.astype(np.float16).copy()
    (Wq1, bq1), (Wq2, bq2), (Wq3, bq3) = P['q']
    shared["w_q1h"] = _pad_w(Wq1[:ATOM_DIM], 2)
    shared["w_q1e"] = _pad_w(Wq1[ATOM_DIM:], 1)
    shared["b_q1"] = bq1.astype(np.float32).reshape(8, 128)
    shared["w_q2"] = _pad_w(Wq2, 8)
    shared["b_q2"] = bq2.astype(np.float32).reshape(8, 128)
    shared["w_q3"] = _pad_w(Wq3 * np.float32(HEAD_DIM ** -0.5), 8)
    shared["b_q3"] = (bq3 * np.float32(HEAD_DIM ** -0.5)).astype(np.float32).reshape(4, 128)
    for nm, plist in [("k", P['k']), ("v", P['v'])]:
        (W1, b1), (W2, b2), (W3, b3) = plist
        shared[f"w_{nm}1"] = _pad_w(W1, 4)
        shared[f"b_{nm}1"] = b1.astype(np.float32).reshape(8, 128)
        shared[f"w_{nm}2"] = _pad_w(W2, 8)
        shared[f"b_{nm}2"] = b2.astype(np.float32).reshape(8, 128)
        shared[f"w_{nm}3"] = _pad_w(W3, 8)
        if nm == "k":
            shared["b_k3"] = b3.astype(np.float32).reshape(4, 128)
        else:
            shared["b_v3"] = b3.astype(np.float32).reshape(NHEADS, 64)
    (Wo1, bo1), (Wo2, bo2) = P['o']
    shared["w_o1"] = _pad_w(Wo1, 4)
    shared["b_o1"] = bo1.astype(np.float32).reshape(8, 128)
    shared["w_o2"] = _pad_w(Wo2, 8)
    shared["b_o2"] = bo2.astype(np.float32).reshape(4, 128)

    in_maps = []
    for c in range(N_CORES):
        bs = slice(c * BPC, (c + 1) * BPC)
        m = dict(shared)
        # atom_staticT: [feat(512 pad), cols=(b_local*128+n)] -> [4,128,512]
        a = atom_static[bs]                       # [4,128,388]
        aT = np.zeros((KSTAT, BPC * N), np.float16)
        aT[:ATOM_STATIC] = a.reshape(BPC * N, ATOM_STATIC).T
        m["atomT"] = aT.reshape(4, 128, BPC * N)
        m["h_absT"] = np.ascontiguousarray(
            h[bs, ai, :].T.astype(np.float16)).reshape(2, 128, BPC)
        m["bias_n"] = np.ascontiguousarray(bias_n[bs])  # [4,128]
        in_maps.append(m)
    return in_maps


class _Runner:
    """Compile once; run the NEFF on 8 cores repeatedly via PJRT shard_map."""

    def __init__(self, repeats=1):
        import jax
        from jax.sharding import Mesh, PartitionSpec
        from jax.experimental.shard_map import shard_map
        from concourse import bass2jax, mybir as _mybir

        nc, _ = _build(repeats)
        self.nc = nc
        bass2jax.install_neuronx_cc_hook()
        in_names, out_names, out_avals, zero_outs = [], [], [], []
        for alloc in nc.m.functions[0].allocations:
            if not isinstance(alloc, _mybir.MemoryLocationSet):
                continue
            name = alloc.memorylocations[0].name
            if alloc.kind == "ExternalInput":
                in_names.append(name)
            elif alloc.kind == "ExternalOutput":
                out_names.append(name)
                shape = tuple(alloc.tensor_shape)
                dtype = _mybir.dt.np(alloc.dtype)
                out_avals.append(jax.core.ShapedArray(shape, dtype))
                zero_outs.append(np.zeros(shape, dtype))
        partition_name = (nc.partition_id_tensor.name
                          if nc.partition_id_tensor else None)
        if partition_name is not None:
            in_names = [n for n in in_names if n != partition_name]
        self.in_names, self.out_names = in_names, out_names
        self.zero_outs = zero_outs
        n_params, n_outs = len(in_names), len(out_names)
        all_in_names = in_names + out_names
        if partition_name is not None:
            all_in_names = all_in_names + [partition_name]

        import jax.numpy as jnp
        from jax.sharding import NamedSharding

        def _body(*args):
            operands = list(args)
            if partition_name is not None:
                operands.append(bass2jax.partition_id_tensor())
            outs = bass2jax._bass_exec_p.bind(
                *operands,
                out_avals=tuple(out_avals),
                in_names=tuple(all_in_names),
                out_names=tuple(out_names),
                lowering_input_output_aliases=(),
                sim_require_finite=True,
                sim_require_nnan=True,
                nc=nc,
            )
            return tuple(outs)

        devices = jax.devices()[:N_CORES]
        mesh = Mesh(np.asarray(devices), ("core",))
        self._fn = jax.jit(
            shard_map(_body, mesh=mesh,
                      in_specs=(PartitionSpec("core"),) * (n_params + n_outs),
                      out_specs=(PartitionSpec("core"),) * n_outs,
                      check_rep=False),
            donate_argnums=tuple(range(n_params, n_params + n_outs)),
            keep_unused=True)
        zshard = NamedSharding(mesh, PartitionSpec("core"))
        self._zeros_fn = jax.jit(
            lambda: tuple(jnp.zeros((N_CORES * z.shape[0], *z.shape[1:]), z.dtype)
                          for z in zero_outs),
            out_shardings=tuple(zshard for _ in zero_outs))

    def concat_inputs(self, in_maps):
        return [np.concatenate([np.asarray(in_maps[c][n]) for c in range(N_CORES)],
                               axis=0) for n in self.in_names]

    def run_raw(self, concat_in):
        return self._fn(*concat_in, *self._zeros_fn())

    def run(self, in_maps):
        out_arrs = self.run_raw(self.concat_inputs(in_maps))
        out = {}
        for i, name in enumerate(self.out_names):
            a = np.asarray(out_arrs[i])
            out[name] = a.reshape(N_CORES, a.shape[0] // N_CORES, *a.shape[1:])
        return out


def _get_runner(repeats=1):
    key = f"runner{repeats}"
    if key not in _CACHE:
        _CACHE[key] = _Runner(repeats)
    return _CACHE[key]


def kernel(h, z, pos, mask, e_feat, params, absorber_index):
    runner = _get_runner()
    in_maps = _prep_maps(h, z, pos, mask, e_feat, params, absorber_index)
    res = runner.run(in_maps)
    o = res["out"]                      # [cores, BPC, 4, 128, NE]
    o = o.reshape(B, LATENT, NE).transpose(0, 2, 1)   # [B, e, latent]
    return np.ascontiguousarray(o.astype(np.float32))
